# revision 6
# baseline (speedup 1.0000x reference)
"""GATv2 (2-layer, N=50000, E=800000) on 8 Trainium2 NeuronCores.

Strategy (self-contained; shapes hardcoded for nn_GATUnit_34067680592302):
  - Nodes partitioned across 8 cores (6250 each, padded to 6272 = 49 blocks
    of 128). Edges (incl. self-loops) assigned by destination node and sorted
    by destination, so scatter-softmax / segment-sum stay core-local.
  - Per layer, every core holds the full "source transform" table
    xl = x @ Wl in its DRAM (layer 1: computed locally from replicated x;
    layer 2: local h shard transformed then AllGather'ed), and bulk-gathers
    xl[src] rows per destination block with ONE dma_gather per block-half
    (int16 indices limit a gather table to 32K rows, so the table is split
    in two 25088-row halves and each block's edge list is stored as
    [half0-edges | pad | half1-edges | pad], each padded to a 128 multiple
    with index-0 rows that are masked out via dst=-1).
  - Per 128-node block, attention + weighted aggregation accumulate in PSUM
    via selection-matrix matmuls; a final reciprocal-scale epilogue divides
    by the softmax denominators (max-subtraction is skipped: |s| <= ~10 so
    exp() is safe in fp32).
"""
import sys
sys.path.insert(0, "/opt/trn_rl_repo")

import numpy as np

NEG = 0.2
USE_PRELU = True  # HW Prelu == leaky_relu(x, alpha); CoreSim lacks it
GATHER_FROM_SHARED = True  # layer-2 gathers read the AllGather buf directly


class Cfg:
    def __init__(self, N=50000, E=800000, ncores=8, nloc=6250, F=128):
        assert N == ncores * nloc
        self.N, self.E, self.ncores, self.nloc, self.F = N, E, ncores, nloc, F
        self.nblk = -(-nloc // 128)          # blocks of 128 nodes per core
        self.nlocp = self.nblk * 128         # padded local nodes
        self.npad = ncores * self.nlocp      # padded global nodes
        self.half = self.npad // 2           # gather-table half size (<32K)
        assert self.npad % 1024 == 0 and self.half < 32768


CFG = Cfg()


# --------------------------------------------------------------------------
# Host-side preprocessing
# --------------------------------------------------------------------------

def host_prep(x, edge_index, Wl1, Wr1, att1, b1, Wl2, Wr2, att2, b2, cfg):
    N, E, NC, NLOC = cfg.N, cfg.E, cfg.ncores, cfg.nloc
    NBLK, NLOCP, NPAD, F = cfg.nblk, cfg.nlocp, cfg.npad, cfg.F
    HALF = cfg.half

    src0 = np.asarray(edge_index[0]).astype(np.int64)
    dst0 = np.asarray(edge_index[1]).astype(np.int64)
    loops = np.arange(N, dtype=np.int64)
    SRC = np.concatenate([src0, loops])
    DST = np.concatenate([dst0, loops])
    shard = DST // NLOC
    src_g = ((SRC // NLOC) * NLOCP + (SRC % NLOC)).astype(np.int32)
    dst_loc = (DST - shard * NLOC).astype(np.int64)

    per_core = []
    n_lo = np.zeros((NC, NBLK), dtype=np.int64)
    n_hi = np.zeros((NC, NBLK), dtype=np.int64)
    for c in range(NC):
        sel = shard == c
        sg, dl = src_g[sel], dst_loc[sel]
        order = np.argsort(dl, kind="stable")
        sg, dl = sg[order], dl[order]
        blk = dl // 128
        lo = sg < HALF
        per_core.append((sg, dl, blk, lo))
        n_lo[c] = np.bincount(blk[lo], minlength=NBLK)
        n_hi[c] = np.bincount(blk[~lo], minlength=NBLK)

    t0_b = (-(-n_lo.max(axis=0) // 128)).astype(np.int64)  # lo tiles per blk
    t1_b = (-(-n_hi.max(axis=0) // 128)).astype(np.int64)  # hi tiles per blk
    tiles_b = t0_b + t1_b
    pcnt = tiles_b * 128
    offs = np.concatenate([[0], np.cumsum(pcnt)])
    NTOT = int(offs[-1])
    NT = NTOT // 128
    PCMAX = int(pcnt.max())

    import ml_dtypes
    bf16 = ml_dtypes.bfloat16

    core_arrays = []
    for c in range(NC):
        sg, dl, blk, lo = per_core[c]
        idx_arr = np.zeros(NTOT, dtype=np.int16)
        dst_arr = np.full(NTOT, -1.0, dtype=np.float32)
        for b in range(NBLK):
            selb = blk == b
            sgb, dlb, lob = sg[selb], dl[selb] - 128 * b, lo[selb]
            o = int(offs[b])
            sl, dll = sgb[lob], dlb[lob]
            idx_arr[o:o + len(sl)] = sl.astype(np.int16)
            dst_arr[o:o + len(sl)] = dll
            o2 = o + int(t0_b[b]) * 128
            sh, dlh = sgb[~lob] - HALF, dlb[~lob]
            idx_arr[o2:o2 + len(sh)] = sh.astype(np.int16)
            dst_arr[o2:o2 + len(sh)] = dlh
        # dma_gather wraps index j to [j % 16, j // 16]; replicate x8 rows
        idx16 = np.tile(
            np.ascontiguousarray(idx_arr.reshape(-1, 16).T), (8, 1))
        dstcol = np.ascontiguousarray(
            dst_arr.reshape(NT, 128).T).astype(bf16)
        dstrow = np.full((NBLK, PCMAX), -1.0, dtype=np.float32)
        for b in range(NBLK):
            o = int(offs[b])
            dstrow[b, :int(pcnt[b])] = dst_arr[o:o + int(pcnt[b])]
        core_arrays.append(dict(idx16=idx16, dstcol=dstcol,
                                dstrow=dstrow.astype(bf16)))

    # node features, transposed + padded: xT[f, g] with g = s*NLOCP + j
    x = np.asarray(x, dtype=np.float32)
    xpad = np.zeros((NPAD, F), dtype=np.float32)
    for s in range(NC):
        xpad[s * NLOCP:s * NLOCP + NLOC] = x[s * NLOC:(s + 1) * NLOC]
    xTfull = np.ascontiguousarray(xpad.T)

    H1 = att1.shape[0]
    C1 = att1.shape[1]
    att1m = np.zeros((128, H1), dtype=np.float32)
    for h in range(H1):
        att1m[h * C1:(h + 1) * C1, h] = att1[h]
    att2m = np.zeros((128, 1), dtype=np.float32)
    att2m[:att2.shape[1], 0] = att2[0]

    iota = np.arange(128, dtype=np.float32)
    padmat = (np.arange(NLOCP) >= NLOC).astype(np.float32).reshape(1, NLOCP)

    shared = dict(
        xTfull=xTfull,
        Wl1=np.asarray(Wl1, np.float32), Wr1=np.asarray(Wr1, np.float32),
        Wl2=np.asarray(Wl2, np.float32), Wr2=np.asarray(Wr2, np.float32),
        att1m=att1m, att2m=att2m,
        bias1r=np.tile(np.asarray(b1, np.float32), (128, 1)),
        bias2r=np.tile(np.asarray(b2, np.float32), (128, 1)),
        id128=np.eye(128, dtype=np.float32),
        iorowb=np.tile(iota, (128, 1)).astype(bf16),
        iocol=iota.reshape(128, 1).copy(),
        iocolb=iota.reshape(128, 1).astype(bf16),
        padmat=padmat,
        e01=np.concatenate([np.ones(H1, np.float32),
                            np.zeros(F, np.float32)]).reshape(1, H1 + F),
        e02=np.concatenate([np.ones(1, np.float32),
                            np.zeros(F, np.float32)]).reshape(1, 1 + F),
    )
    in_maps = []
    for c in range(NC):
        m = dict(shared)
        m["xTloc"] = np.ascontiguousarray(xTfull[:, c * NLOCP:(c + 1) * NLOCP])
        m.update(core_arrays[c])
        in_maps.append(m)
    meta = dict(pcnt=pcnt, tiles_b=tiles_b, t0_b=t0_b, t1_b=t1_b,
                NT=NT, H1=H1, pcmax=PCMAX)
    return in_maps, meta


# --------------------------------------------------------------------------
# Device program
# --------------------------------------------------------------------------

def build_nc(cfg, meta, profile_nocc=False):
    import concourse.bacc as bacc
    import concourse.tile as tile
    from concourse import mybir

    f32 = mybir.dt.float32
    bf16 = mybir.dt.bfloat16
    i16 = mybir.dt.int16
    AF = mybir.ActivationFunctionType
    OP = mybir.AluOpType

    NC, F = cfg.ncores, cfg.F
    NBLK, NLOCP, NPAD, HALF = cfg.nblk, cfg.nlocp, cfg.npad, cfg.half
    NT, H1 = meta["NT"], meta["H1"]
    tiles_b, t0_b, t1_b = meta["tiles_b"], meta["t0_b"], meta["t1_b"]
    PCMAX = meta["pcmax"]
    TBMAX = PCMAX // 128

    nc = bacc.Bacc("TRN2", target_bir_lowering=False)

    din = {}
    def ein(name, shape, dt=f32):
        din[name] = nc.dram_tensor(name, shape, dt, kind="ExternalInput")
        return din[name]

    d_xTfull = ein("xTfull", [128, NPAD])
    d_xTloc = ein("xTloc", [128, NLOCP])
    d_Wl1, d_Wr1 = ein("Wl1", [128, 128]), ein("Wr1", [128, 128])
    d_Wl2, d_Wr2 = ein("Wl2", [128, 128]), ein("Wr2", [128, 128])
    d_att1, d_att2 = ein("att1m", [128, H1]), ein("att2m", [128, 1])
    d_b1r, d_b2r = ein("bias1r", [128, F]), ein("bias2r", [128, F])
    d_id = ein("id128", [128, 128])
    d_iorowb = ein("iorowb", [128, 128], bf16)
    d_iocol = ein("iocol", [128, 1])
    d_iocolb = ein("iocolb", [128, 1], bf16)
    d_padm = ein("padmat", [1, NLOCP])
    d_e01, d_e02 = ein("e01", [1, H1 + F]), ein("e02", [1, 1 + F])
    d_idx16 = ein("idx16", [128, NT * 8], i16)
    d_dstcol = ein("dstcol", [128, NT], bf16)
    d_dstrow = ein("dstrow", [NBLK, PCMAX], bf16)

    d_out = nc.dram_tensor("outloc", [NLOCP, F], f32, kind="ExternalOutput")

    d_xl1lo = nc.dram_tensor("xl1lo", [HALF, F], f32)
    d_xl1hi = nc.dram_tensor("xl1hi", [HALF, F], f32)
    d_xl2sh = nc.dram_tensor("xl2sh", [NPAD, F], f32, addr_space="Shared")
    d_xl2loc = nc.dram_tensor("xl2loc", [NLOCP, F], f32)
    if not GATHER_FROM_SHARED:
        d_xl2lo = nc.dram_tensor("xl2lo", [HALF, F], f32)
        d_xl2hi = nc.dram_tensor("xl2hi", [HALF, F], f32)

    with tile.TileContext(nc) as tc:
        with tc.tile_pool(name="const", bufs=1) as cp:
            Wl1_sb = cp.tile_from(d_Wl1[:, :])
            Wr1_sb = cp.tile_from(d_Wr1[:, :])
            Wl2_sb = cp.tile_from(d_Wl2[:, :])
            Wr2_sb = cp.tile_from(d_Wr2[:, :])
            att1_sb = cp.tile_from(d_att1[:, :])
            att2_sb = cp.tile_from(d_att2[:, :])
            b1_sb = cp.tile_from(d_b1r[:, :])
            b2_sb = cp.tile_from(d_b2r[:, :])
            id_sb = cp.tile_from(d_id[:, :])
            iorowb_sb = cp.tile_from(d_iorowb[:, :])
            iocol_sb = cp.tile_from(d_iocol[:, :])
            iocolb_sb = cp.tile_from(d_iocolb[:, :])
            padm_sb = cp.tile_from(d_padm[:, :])
            e01_sb = cp.tile_from(d_e01[:, :])
            e02_sb = cp.tile_from(d_e02[:, :])
            idx_sb = cp.tile_from(d_idx16[:, :])
            dstc_sb = cp.tile_from(d_dstcol[:, :])
            xr1h_sb = cp.tile([128, NLOCP], bf16)
            xr1l_sb = cp.tile([128, NLOCP], bf16)
            xr2h_sb = cp.tile([128, NLOCP], bf16)
            xr2l_sb = cp.tile([128, NLOCP], bf16)

            # ---------------- phase A: layer-1 tables ----------------
            with (
                tc.tile_pool(name="tabs", bufs=4) as tp,
                tc.tile_pool(name="tabp", bufs=2, space="PSUM") as tpp,
            ):
                for t4 in range(-(-NBLK // 4)):  # local-shard xr1
                    q = min(4, NBLK - 4 * t4)
                    xt = tp.tile([128, 512], f32, tag="xt")
                    nc.scalar.dma_start(
                        out=xt[:, 0:q * 128],
                        in_=d_xTloc[:, t4 * 512:t4 * 512 + q * 128])
                    for j in range(q):
                        t = 4 * t4 + j
                        ps = tpp.tile([128, 128], f32, tag="psx")
                        nc.tensor.matmul(out=ps[:],
                                         lhsT=xt[:, j * 128:(j + 1) * 128],
                                         rhs=Wr1_sb[:], start=True, stop=True)
                        nc.scalar.copy(xr1h_sb[:, t * 128:(t + 1) * 128], ps[:])
                        nc.vector.tensor_tensor(
                            out=xr1l_sb[:, t * 128:(t + 1) * 128], in0=ps[:],
                            in1=xr1h_sb[:, t * 128:(t + 1) * 128],
                            op=OP.subtract)
                for t4 in range(NPAD // 512):  # full xl1 table locally
                    ps4 = tpp.tile([128, 512], f32, tag="ps4")
                    xt = tp.tile([128, 512], f32, tag="xt")
                    nc.scalar.dma_start(out=xt[:],
                                        in_=d_xTfull[:, t4 * 512:(t4 + 1) * 512])
                    for j in range(4):
                        nc.tensor.matmul(out=ps4[:, j * 128:(j + 1) * 128],
                                         lhsT=xt[:, j * 128:(j + 1) * 128],
                                         rhs=Wl1_sb[:],
                                         start=(j == 0), stop=(j == 3),
                                         skip_group_check=True)
                    stg = tp.tile([128, 512], f32, tag="stg")
                    nc.scalar.copy(stg[:], ps4[:])
                    half_t = NPAD // 1024  # 512-row chunks per half table
                    d_tab = d_xl1lo if t4 < half_t else d_xl1hi
                    r0 = (t4 if t4 < half_t else t4 - half_t) * 512
                    nc.sync.dma_start(
                        out=d_tab[r0:r0 + 512, :]
                            .rearrange("(t p) f -> p t f", p=128),
                        in_=stg[:].rearrange("p (t f) -> p t f", t=4),
                    )

            # ---------------- phase B: edge layers ----------------
            def edge_layer(H, d_lo, d_hi, xr_hi, xr_lo, att_sb, e0_sb,
                           bias_sb, epilogue):
                with (
                    tc.tile_pool(name="bp", bufs=2) as bp,
                    tc.tile_pool(name="gp", bufs=3) as gp,
                    tc.tile_pool(name="op", bufs=2) as op,
                    tc.tile_pool(name="epp", bufs=2, space="PSUM") as pp,
                    tc.tile_pool(name="epp1", bufs=1, space="PSUM") as pp1,
                ):
                    ct = 0
                    for b in range(NBLK):
                        T0, T1 = int(t0_b[b]), int(t1_b[b])
                        TB = T0 + T1
                        EB = TB * 128
                        b_acc = pp.tile([128, H + F], f32, tag="b_acc")
                        nc.tensor.matmul(out=b_acc[:],
                                         lhsT=padm_sb[0:1, b * 128:(b + 1) * 128],
                                         rhs=e0_sb[0:1, 0:H + F],
                                         start=True, stop=False,
                                         skip_group_check=True)
                        drst = bp.tile([1, PCMAX], bf16, tag="drst")
                        nc.scalar.dma_start(out=drst[0:1, 0:EB],
                                            in_=d_dstrow[b:b + 1, 0:EB])
                        # bulk gather of xl[src] for the whole block (one
                        # dma_gather per table half)
                        xl_g = bp.tile([128, TBMAX * F], f32, tag="xl_g")
                        if T0 > 0:
                            nc.gpsimd.dma_gather(
                                out_ap=xl_g[:, 0:T0 * F]
                                    .rearrange("p (t f) -> p t f", t=T0),
                                in_ap=d_lo,
                                idxs_ap=idx_sb[:, 8 * ct:8 * (ct + T0)],
                                num_idxs=T0 * 128,
                                num_idxs_reg=T0 * 128,
                                elem_size=F,
                            )
                        if T1 > 0:
                            nc.gpsimd.dma_gather(
                                out_ap=xl_g[:, T0 * F:TB * F]
                                    .rearrange("p (t f) -> p t f", t=T1),
                                in_ap=d_hi,
                                idxs_ap=idx_sb[:, 8 * (ct + T0):8 * (ct + TB)],
                                num_idxs=T1 * 128,
                                num_idxs_reg=T1 * 128,
                                elem_size=F,
                            )
                        dstrep = bp.tile([128, PCMAX], bf16, tag="dstrep")
                        nc.gpsimd.partition_broadcast(
                            dstrep[:, 0:EB], drst[0:1, 0:EB])
                        m2 = bp.tile([128, PCMAX], bf16, tag="m2")
                        nc.vector.tensor_tensor(
                            out=m2[:, 0:EB], in0=dstrep[:, 0:EB],
                            in1=iocolb_sb[:].to_broadcast([128, EB]),
                            op=OP.is_equal)
                        m = bp.tile([128, PCMAX], bf16, tag="m")
                        nc.vector.tensor_tensor(
                            out=m[:, 0:EB].rearrange("p (t n) -> p t n", t=TB),
                            in0=dstc_sb[:, ct:ct + TB]
                                .unsqueeze(2).to_broadcast([128, TB, 128]),
                            in1=iorowb_sb[:].unsqueeze(1)
                                .to_broadcast([128, TB, 128]),
                            op=OP.is_equal)
                        for g0 in range(0, TB, 4):
                            sz = min(4, TB - g0)
                            E1 = sz * 128
                            o1 = g0 * 128
                            b_et = pp.tile([128, 512], f32, tag="b_et")
                            for j in range(sz):
                                nc.tensor.matmul(
                                    out=b_et[:, j * 128:(j + 1) * 128],
                                    lhsT=xl_g[:, (g0 + j) * F:(g0 + j + 1) * F],
                                    rhs=id_sb[:], is_transpose=True,
                                    start=(j == 0), stop=False,
                                    skip_group_check=True)
                            nc.tensor.matmul(
                                out=b_et[:, 0:E1],
                                lhsT=xr_hi[:, b * 128:(b + 1) * 128],
                                rhs=m2[:, o1:o1 + E1],
                                start=False, stop=False, skip_group_check=True)
                            nc.tensor.matmul(
                                out=b_et[:, 0:E1],
                                lhsT=xr_lo[:, b * 128:(b + 1) * 128],
                                rhs=m2[:, o1:o1 + E1],
                                start=False, stop=True, skip_group_check=True)
                            lrel = gp.tile([128, 512], f32, tag="lrel")
                            if USE_PRELU:
                                nc.scalar.activation(
                                    out=lrel[:, 0:E1], in_=b_et[:, 0:E1],
                                    func=AF.Prelu, alpha=NEG)
                            else:
                                relu = gp.tile([128, 512], f32, tag="relu")
                                nc.scalar.activation(
                                    out=relu[:, 0:E1], in_=b_et[:, 0:E1],
                                    func=AF.Relu, scale=1.0 - NEG)
                                nc.vector.scalar_tensor_tensor(
                                    out=lrel[:, 0:E1], in0=b_et[:, 0:E1],
                                    scalar=NEG, in1=relu[:, 0:E1],
                                    op0=OP.mult, op1=OP.add)
                            b_s = pp.tile([128, 4 * H], f32, tag="b_s")
                            for j in range(sz):
                                nc.tensor.matmul(
                                    out=b_s[:, j * H:(j + 1) * H],
                                    lhsT=lrel[:, j * 128:(j + 1) * 128],
                                    rhs=att_sb[:, 0:H],
                                    start=(j == 0), stop=(j == sz - 1),
                                    skip_group_check=True)
                            w_exp = gp.tile([128, 4 * H], f32, tag="w_exp")
                            nc.scalar.activation(
                                out=w_exp[:, 0:sz * H], in_=b_s[:, 0:sz * H],
                                func=AF.Exp)
                            w_sb = gp.tile([128, 4 * (H + F)], bf16, tag="w_sb")
                            nc.vector.tensor_scalar_mul(
                                w_sb[:].rearrange("p (t x) -> p t x",
                                                  t=4)[:, 0:sz, 0:H],
                                w_exp[:, 0:sz * H]
                                    .rearrange("p (t h) -> p t h", t=sz),
                                1.0)
                            nc.vector.tensor_tensor(
                                out=w_sb[:].rearrange("p (t x) -> p t x",
                                                      t=4)[:, 0:sz, H:H + F]
                                    .rearrange("p t (h c) -> p t h c", h=H),
                                in0=xl_g[:, g0 * F:(g0 + sz) * F]
                                    .rearrange("p (t h c) -> p t h c",
                                               t=sz, h=H),
                                in1=w_exp[:, 0:sz * H]
                                    .rearrange("p (t h) -> p t h", t=sz)
                                    .unsqueeze(3)
                                    .to_broadcast([128, sz, H, F // H]),
                                op=OP.mult)
                            for j in range(sz):
                                nc.tensor.matmul(
                                    out=b_acc[:],
                                    lhsT=m[:, (g0 + j) * 128:(g0 + j + 1) * 128],
                                    rhs=w_sb[:, j * (H + F):(j + 1) * (H + F)],
                                    start=False,
                                    stop=(g0 + sz >= TB and j == sz - 1),
                                    skip_group_check=True)
                        ct += TB
                        # epilogue: divide by denominators, add bias
                        recip = op.tile([128, H], f32, tag="recip")
                        nc.vector.reciprocal(recip[:], b_acc[:, 0:H])
                        outb = op.tile([128, F], f32, tag="outb")
                        C = F // H
                        for h in range(H):
                            nc.vector.tensor_scalar_mul(
                                outb[:, h * C:(h + 1) * C],
                                b_acc[:, H + h * C:H + (h + 1) * C],
                                recip[:, h:h + 1])
                        nc.vector.tensor_tensor(out=outb[:], in0=outb[:],
                                                in1=bias_sb[:], op=OP.add)
                        epilogue(b, outb, op, pp1)

            def epi1(b, outb, wp, pp1):
                ps_h = pp1.tile([128, 128], f32, tag="ps_h")
                nc.tensor.matmul(out=ps_h[:], lhsT=outb[:], rhs=id_sb[:],
                                 is_transpose=True, start=True, stop=True)
                hT = wp.tile([128, 128], f32, tag="hT")
                nc.scalar.copy(hT[:], ps_h[:])
                ps_x = pp1.tile([128, 128], f32, tag="ps_x2")
                nc.tensor.matmul(out=ps_x[:], lhsT=hT[:], rhs=Wr2_sb[:],
                                 start=True, stop=True)
                nc.scalar.copy(xr2h_sb[:, b * 128:(b + 1) * 128], ps_x[:])
                nc.vector.tensor_tensor(
                    out=xr2l_sb[:, b * 128:(b + 1) * 128], in0=ps_x[:],
                    in1=xr2h_sb[:, b * 128:(b + 1) * 128], op=OP.subtract)
                ps_l = pp1.tile([128, 128], f32, tag="ps_x2")
                nc.tensor.matmul(out=ps_l[:], lhsT=hT[:], rhs=Wl2_sb[:],
                                 start=True, stop=True)
                l2s = wp.tile([128, 128], f32, tag="l2s")
                nc.scalar.copy(l2s[:], ps_l[:])
                nc.sync.dma_start(out=d_xl2loc[b * 128:(b + 1) * 128, :],
                                  in_=l2s[:])

            edge_layer(H1, d_xl1lo[:, :], d_xl1hi[:, :], xr1h_sb, xr1l_sb,
                       att1_sb, e01_sb, b1_sb, epi1)

            # ------------- phase C: AllGather layer-2 table -------------
            if profile_nocc:
                for s in range(NC):
                    nc.sync.dma_start(
                        out=d_xl2sh[s * NLOCP:(s + 1) * NLOCP, :],
                        in_=d_xl2loc[:, :])
            else:
                nc.gpsimd.collective_compute(
                    "AllGather", mybir.AluOpType.bypass,
                    replica_groups=[list(range(NC))],
                    ins=[d_xl2loc[:, :]], outs=[d_xl2sh[:, :]],
                )
            if GATHER_FROM_SHARED:
                d_l2lo, d_l2hi = d_xl2sh[0:HALF, :], d_xl2sh[HALF:NPAD, :]
            else:
                nc.sync.dma_start(out=d_xl2lo[:, :], in_=d_xl2sh[0:HALF, :])
                nc.scalar.dma_start(out=d_xl2hi[:, :],
                                    in_=d_xl2sh[HALF:NPAD, :])
                d_l2lo, d_l2hi = d_xl2lo[:, :], d_xl2hi[:, :]

            # ---------------- phase E: layer-2 edges ----------------
            def epi2(b, outb, wp, pp1):
                nc.sync.dma_start(out=d_out[b * 128:(b + 1) * 128, :],
                                  in_=outb[:])

            edge_layer(1, d_l2lo, d_l2hi, xr2h_sb, xr2l_sb, att2_sb,
                       e02_sb, b2_sb, epi2)

    nc.compile()
    return nc


# --------------------------------------------------------------------------
# Entry point
# --------------------------------------------------------------------------

_NC_CACHE = {}


def kernel(x, edge_index, edge_attr, Wl1, Wr1, att1, b1, Wl2, Wr2, att2, b2,
           cfg=None, _want_results=False):
    from concourse.bass_utils import run_bass_kernel_spmd

    cfg = cfg or CFG
    in_maps, meta = host_prep(x, edge_index, Wl1, Wr1, att1, b1,
                              Wl2, Wr2, att2, b2, cfg)
    key = (cfg.N, cfg.E, tuple(meta["pcnt"].tolist()))
    nc = _NC_CACHE.get(key)
    if nc is None:
        nc = build_nc(cfg, meta)
        _NC_CACHE[key] = nc
    res = run_bass_kernel_spmd(nc, in_maps, core_ids=list(range(cfg.ncores)))
    out = np.empty((cfg.N, cfg.F), dtype=np.float32)
    for c in range(cfg.ncores):
        out[c * cfg.nloc:(c + 1) * cfg.nloc] = \
            res.results[c]["outloc"][:cfg.nloc]
    if _want_results:
        return out, res
    return out


# revision 7
# speedup vs baseline: 1.1449x; 1.1449x over previous
"""GATv2 (2-layer, N=50000, E=800000) on 8 Trainium2 NeuronCores.

Strategy (self-contained; shapes hardcoded for nn_GATUnit_34067680592302):
  - Nodes partitioned across 8 cores (6250 each, padded to 6272 = 49 blocks
    of 128). Edges (incl. self-loops) assigned by destination node and sorted
    by destination, so scatter-softmax / segment-sum stay core-local.
  - Per layer, every core holds the full "source transform" table
    xl = x @ Wl in its DRAM (layer 1: computed locally from replicated x;
    layer 2: local h shard transformed then AllGather'ed), and bulk-gathers
    xl[src] rows per destination block with ONE dma_gather per block-half
    (int16 indices limit a gather table to 32K rows, so the table is split
    in two 25088-row halves and each block's edge list is stored as
    [half0-edges | pad | half1-edges | pad], each padded to a 128 multiple
    with index-0 rows that are masked out via dst=-1).
  - Per 128-node block, attention + weighted aggregation accumulate in PSUM
    via selection-matrix matmuls; a final reciprocal-scale epilogue divides
    by the softmax denominators (max-subtraction is skipped: |s| <= ~10 so
    exp() is safe in fp32).
"""
import sys
sys.path.insert(0, "/opt/trn_rl_repo")

import numpy as np

NEG = 0.2
USE_PRELU = True  # HW Prelu == leaky_relu(x, alpha); CoreSim lacks it
GATHER_FROM_SHARED = True  # layer-2 gathers read the AllGather buf directly


class Cfg:
    def __init__(self, N=50000, E=800000, ncores=8, nloc=6250, F=128):
        assert N == ncores * nloc
        self.N, self.E, self.ncores, self.nloc, self.F = N, E, ncores, nloc, F
        self.nblk = -(-nloc // 128)          # blocks of 128 nodes per core
        self.nlocp = self.nblk * 128         # padded local nodes
        self.npad = ncores * self.nlocp      # padded global nodes
        self.half = self.npad // 2           # gather-table half size (<32K)
        assert self.npad % 1024 == 0 and self.half < 32768


CFG = Cfg()


# --------------------------------------------------------------------------
# Host-side preprocessing
# --------------------------------------------------------------------------

def host_prep(x, edge_index, Wl1, Wr1, att1, b1, Wl2, Wr2, att2, b2, cfg):
    N, E, NC, NLOC = cfg.N, cfg.E, cfg.ncores, cfg.nloc
    NBLK, NLOCP, NPAD, F = cfg.nblk, cfg.nlocp, cfg.npad, cfg.F
    HALF = cfg.half

    src0 = np.asarray(edge_index[0]).astype(np.int64)
    dst0 = np.asarray(edge_index[1]).astype(np.int64)
    loops = np.arange(N, dtype=np.int64)
    SRC = np.concatenate([src0, loops])
    DST = np.concatenate([dst0, loops])
    shard = DST // NLOC
    src_g = ((SRC // NLOC) * NLOCP + (SRC % NLOC)).astype(np.int32)
    dst_loc = (DST - shard * NLOC).astype(np.int64)

    per_core = []
    n_lo = np.zeros((NC, NBLK), dtype=np.int64)
    n_hi = np.zeros((NC, NBLK), dtype=np.int64)
    for c in range(NC):
        sel = shard == c
        sg, dl = src_g[sel], dst_loc[sel]
        order = np.argsort(dl, kind="stable")
        sg, dl = sg[order], dl[order]
        blk = dl // 128
        lo = sg < HALF
        per_core.append((sg, dl, blk, lo))
        n_lo[c] = np.bincount(blk[lo], minlength=NBLK)
        n_hi[c] = np.bincount(blk[~lo], minlength=NBLK)

    t0_b = (-(-n_lo.max(axis=0) // 128)).astype(np.int64)  # lo tiles per blk
    t1_b = (-(-n_hi.max(axis=0) // 128)).astype(np.int64)  # hi tiles per blk
    tiles_b = t0_b + t1_b
    pcnt = tiles_b * 128
    offs = np.concatenate([[0], np.cumsum(pcnt)])
    NTOT = int(offs[-1])
    NT = NTOT // 128
    PCMAX = int(pcnt.max())

    import ml_dtypes
    bf16 = ml_dtypes.bfloat16

    core_arrays = []
    for c in range(NC):
        sg, dl, blk, lo = per_core[c]
        idx_arr = np.zeros(NTOT, dtype=np.int16)
        dst_arr = np.full(NTOT, -1.0, dtype=np.float32)
        for b in range(NBLK):
            selb = blk == b
            sgb, dlb, lob = sg[selb], dl[selb] - 128 * b, lo[selb]
            o = int(offs[b])
            sl, dll = sgb[lob], dlb[lob]
            idx_arr[o:o + len(sl)] = sl.astype(np.int16)
            dst_arr[o:o + len(sl)] = dll
            o2 = o + int(t0_b[b]) * 128
            sh, dlh = sgb[~lob] - HALF, dlb[~lob]
            idx_arr[o2:o2 + len(sh)] = sh.astype(np.int16)
            dst_arr[o2:o2 + len(sh)] = dlh
        # dma_gather wraps index j to [j % 16, j // 16]; replicate x8 rows
        idx16 = np.tile(
            np.ascontiguousarray(idx_arr.reshape(-1, 16).T), (8, 1))
        dstcol = np.ascontiguousarray(
            dst_arr.reshape(NT, 128).T).astype(bf16)
        dstrow = np.full((NBLK, PCMAX), -1.0, dtype=np.float32)
        for b in range(NBLK):
            o = int(offs[b])
            dstrow[b, :int(pcnt[b])] = dst_arr[o:o + int(pcnt[b])]
        core_arrays.append(dict(idx16=idx16, dstcol=dstcol,
                                dstrow=dstrow.astype(bf16)))

    # node features, transposed + padded: xT[f, g] with g = s*NLOCP + j
    x = np.asarray(x, dtype=np.float32)
    xpad = np.zeros((NPAD, F), dtype=np.float32)
    for s in range(NC):
        xpad[s * NLOCP:s * NLOCP + NLOC] = x[s * NLOC:(s + 1) * NLOC]
    xTfull = np.ascontiguousarray(xpad.T)

    H1 = att1.shape[0]
    C1 = att1.shape[1]
    att1m = np.zeros((128, H1), dtype=np.float32)
    for h in range(H1):
        att1m[h * C1:(h + 1) * C1, h] = att1[h]
    att2m = np.zeros((128, 1), dtype=np.float32)
    att2m[:att2.shape[1], 0] = att2[0]

    iota = np.arange(128, dtype=np.float32)
    padmat = (np.arange(NLOCP) >= NLOC).astype(np.float32).reshape(1, NLOCP)

    shared = dict(
        xTfull=xTfull,
        Wl1=np.asarray(Wl1, np.float32), Wr1=np.asarray(Wr1, np.float32),
        Wl2=np.asarray(Wl2, np.float32), Wr2=np.asarray(Wr2, np.float32),
        att1m=att1m, att2m=att2m,
        bias1r=np.tile(np.asarray(b1, np.float32), (128, 1)),
        bias2r=np.tile(np.asarray(b2, np.float32), (128, 1)),
        id128=np.eye(128, dtype=np.float32),
        iorowb=np.tile(iota, (128, 1)).astype(bf16),
        iocol=iota.reshape(128, 1).copy(),
        iocolb=iota.reshape(128, 1).astype(bf16),
        padmat=padmat,
        e01=np.concatenate([np.ones(H1, np.float32),
                            np.zeros(F, np.float32)]).reshape(1, H1 + F),
        e02=np.concatenate([np.ones(1, np.float32),
                            np.zeros(F, np.float32)]).reshape(1, 1 + F),
    )
    in_maps = []
    for c in range(NC):
        m = dict(shared)
        m["xTloc"] = np.ascontiguousarray(xTfull[:, c * NLOCP:(c + 1) * NLOCP])
        m.update(core_arrays[c])
        in_maps.append(m)
    meta = dict(pcnt=pcnt, tiles_b=tiles_b, t0_b=t0_b, t1_b=t1_b,
                NT=NT, H1=H1, pcmax=PCMAX)
    return in_maps, meta


# --------------------------------------------------------------------------
# Device program
# --------------------------------------------------------------------------

def build_nc(cfg, meta, profile_nocc=False):
    import concourse.bacc as bacc
    import concourse.tile as tile
    from concourse import mybir

    f32 = mybir.dt.float32
    bf16 = mybir.dt.bfloat16
    i16 = mybir.dt.int16
    AF = mybir.ActivationFunctionType
    OP = mybir.AluOpType

    NC, F = cfg.ncores, cfg.F
    NBLK, NLOCP, NPAD, HALF = cfg.nblk, cfg.nlocp, cfg.npad, cfg.half
    NT, H1 = meta["NT"], meta["H1"]
    tiles_b, t0_b, t1_b = meta["tiles_b"], meta["t0_b"], meta["t1_b"]
    PCMAX = meta["pcmax"]
    TBMAX = PCMAX // 128

    nc = bacc.Bacc("TRN2", target_bir_lowering=False)

    din = {}
    def ein(name, shape, dt=f32):
        din[name] = nc.dram_tensor(name, shape, dt, kind="ExternalInput")
        return din[name]

    d_xTfull = ein("xTfull", [128, NPAD])
    d_xTloc = ein("xTloc", [128, NLOCP])
    d_Wl1, d_Wr1 = ein("Wl1", [128, 128]), ein("Wr1", [128, 128])
    d_Wl2, d_Wr2 = ein("Wl2", [128, 128]), ein("Wr2", [128, 128])
    d_att1, d_att2 = ein("att1m", [128, H1]), ein("att2m", [128, 1])
    d_b1r, d_b2r = ein("bias1r", [128, F]), ein("bias2r", [128, F])
    d_id = ein("id128", [128, 128])
    d_iorowb = ein("iorowb", [128, 128], bf16)
    d_iocol = ein("iocol", [128, 1])
    d_iocolb = ein("iocolb", [128, 1], bf16)
    d_padm = ein("padmat", [1, NLOCP])
    d_e01, d_e02 = ein("e01", [1, H1 + F]), ein("e02", [1, 1 + F])
    d_idx16 = ein("idx16", [128, NT * 8], i16)
    d_dstcol = ein("dstcol", [128, NT], bf16)
    d_dstrow = ein("dstrow", [NBLK, PCMAX], bf16)

    d_out = nc.dram_tensor("outloc", [NLOCP, F], f32, kind="ExternalOutput")

    d_xl1lo = nc.dram_tensor("xl1lo", [HALF, F], f32)
    d_xl1hi = nc.dram_tensor("xl1hi", [HALF, F], f32)
    d_xl2sh = nc.dram_tensor("xl2sh", [NPAD, F], f32, addr_space="Shared")
    d_xl2loc = nc.dram_tensor("xl2loc", [NLOCP, F], f32)
    if not GATHER_FROM_SHARED:
        d_xl2lo = nc.dram_tensor("xl2lo", [HALF, F], f32)
        d_xl2hi = nc.dram_tensor("xl2hi", [HALF, F], f32)

    with tile.TileContext(nc) as tc:
        with tc.tile_pool(name="const", bufs=1) as cp:
            Wl1_sb = cp.tile_from(d_Wl1[:, :])
            Wr1_sb = cp.tile_from(d_Wr1[:, :])
            Wl2_sb = cp.tile_from(d_Wl2[:, :])
            Wr2_sb = cp.tile_from(d_Wr2[:, :])
            att1_sb = cp.tile_from(d_att1[:, :])
            att2_sb = cp.tile_from(d_att2[:, :])
            b1_sb = cp.tile_from(d_b1r[:, :])
            b2_sb = cp.tile_from(d_b2r[:, :])
            id_sb = cp.tile_from(d_id[:, :])
            iorowb_sb = cp.tile_from(d_iorowb[:, :])
            iocol_sb = cp.tile_from(d_iocol[:, :])
            iocolb_sb = cp.tile_from(d_iocolb[:, :])
            padm_sb = cp.tile_from(d_padm[:, :])
            e01_sb = cp.tile_from(d_e01[:, :])
            e02_sb = cp.tile_from(d_e02[:, :])
            idx_sb = cp.tile_from(d_idx16[:, :])
            dstc_sb = cp.tile_from(d_dstcol[:, :])
            xr1h_sb = cp.tile([128, NLOCP], bf16)
            xr1l_sb = cp.tile([128, NLOCP], bf16)
            xr2h_sb = cp.tile([128, NLOCP], bf16)
            xr2l_sb = cp.tile([128, NLOCP], bf16)

            # ---------------- phase A: layer-1 tables ----------------
            with (
                tc.tile_pool(name="tabs", bufs=4) as tp,
                tc.tile_pool(name="tabp", bufs=2, space="PSUM") as tpp,
            ):
                for t4 in range(-(-NBLK // 4)):  # local-shard xr1
                    q = min(4, NBLK - 4 * t4)
                    xt = tp.tile([128, 512], f32, tag="xt")
                    nc.scalar.dma_start(
                        out=xt[:, 0:q * 128],
                        in_=d_xTloc[:, t4 * 512:t4 * 512 + q * 128])
                    for j in range(q):
                        t = 4 * t4 + j
                        ps = tpp.tile([128, 128], f32, tag="psx")
                        nc.tensor.matmul(out=ps[:],
                                         lhsT=xt[:, j * 128:(j + 1) * 128],
                                         rhs=Wr1_sb[:], start=True, stop=True)
                        nc.scalar.copy(xr1h_sb[:, t * 128:(t + 1) * 128], ps[:])
                        nc.vector.tensor_tensor(
                            out=xr1l_sb[:, t * 128:(t + 1) * 128], in0=ps[:],
                            in1=xr1h_sb[:, t * 128:(t + 1) * 128],
                            op=OP.subtract)
                for t4 in range(NPAD // 512):  # full xl1 table locally
                    ps4 = tpp.tile([128, 512], f32, tag="ps4")
                    xt = tp.tile([128, 512], f32, tag="xt")
                    nc.scalar.dma_start(out=xt[:],
                                        in_=d_xTfull[:, t4 * 512:(t4 + 1) * 512])
                    for j in range(4):
                        nc.tensor.matmul(out=ps4[:, j * 128:(j + 1) * 128],
                                         lhsT=xt[:, j * 128:(j + 1) * 128],
                                         rhs=Wl1_sb[:],
                                         start=(j == 0), stop=(j == 3),
                                         skip_group_check=True)
                    stg = tp.tile([128, 512], f32, tag="stg")
                    nc.scalar.copy(stg[:], ps4[:])
                    half_t = NPAD // 1024  # 512-row chunks per half table
                    d_tab = d_xl1lo if t4 < half_t else d_xl1hi
                    r0 = (t4 if t4 < half_t else t4 - half_t) * 512
                    nc.sync.dma_start(
                        out=d_tab[r0:r0 + 512, :]
                            .rearrange("(t p) f -> p t f", p=128),
                        in_=stg[:].rearrange("p (t f) -> p t f", t=4),
                    )

            # ---------------- phase B: edge layers ----------------
            def edge_layer(H, d_lo, d_hi, xr_hi, xr_lo, att_sb, e0_sb,
                           bias_sb, epilogue):
                with (
                    tc.tile_pool(name="bp", bufs=2) as bp,
                    tc.tile_pool(name="gp", bufs=3) as gp,
                    tc.tile_pool(name="op", bufs=2) as op,
                    tc.tile_pool(name="epp", bufs=2, space="PSUM") as pp,
                    tc.tile_pool(name="epp1", bufs=1, space="PSUM") as pp1,
                ):
                    ct = 0
                    for b in range(NBLK):
                        T0, T1 = int(t0_b[b]), int(t1_b[b])
                        TB = T0 + T1
                        EB = TB * 128
                        b_acc = pp.tile([128, H + F], f32, tag="b_acc")
                        nc.tensor.matmul(out=b_acc[:],
                                         lhsT=padm_sb[0:1, b * 128:(b + 1) * 128],
                                         rhs=e0_sb[0:1, 0:H + F],
                                         start=True, stop=False,
                                         skip_group_check=True)
                        drst = bp.tile([1, PCMAX], bf16, tag="drst")
                        nc.scalar.dma_start(out=drst[0:1, 0:EB],
                                            in_=d_dstrow[b:b + 1, 0:EB])
                        # bulk gather of xl[src] for the whole block (one
                        # dma_gather per table half)
                        xl_g = bp.tile([128, TBMAX * F], f32, tag="xl_g")
                        # dma_gather breaks on HW above 1024 indices; chunk
                        GMAX = 8  # tiles per gather
                        for tab, ta, tb in ((d_lo, 0, T0), (d_hi, T0, TB)):
                            for c0 in range(ta, tb, GMAX):
                                tn = min(GMAX, tb - c0)
                                nc.gpsimd.dma_gather(
                                    out_ap=xl_g[:, c0 * F:(c0 + tn) * F]
                                        .rearrange("p (t f) -> p t f", t=tn),
                                    in_ap=tab,
                                    idxs_ap=idx_sb[:, 8 * (ct + c0):
                                                   8 * (ct + c0 + tn)],
                                    num_idxs=tn * 128,
                                    num_idxs_reg=tn * 128,
                                    elem_size=F,
                                )
                        dstrep = bp.tile([128, PCMAX], bf16, tag="dstrep")
                        nc.gpsimd.partition_broadcast(
                            dstrep[:, 0:EB], drst[0:1, 0:EB])
                        m2 = bp.tile([128, PCMAX], bf16, tag="m2")
                        nc.vector.tensor_tensor(
                            out=m2[:, 0:EB], in0=dstrep[:, 0:EB],
                            in1=iocolb_sb[:].to_broadcast([128, EB]),
                            op=OP.is_equal)
                        m = bp.tile([128, PCMAX], bf16, tag="m")
                        nc.vector.tensor_tensor(
                            out=m[:, 0:EB].rearrange("p (t n) -> p t n", t=TB),
                            in0=dstc_sb[:, ct:ct + TB]
                                .unsqueeze(2).to_broadcast([128, TB, 128]),
                            in1=iorowb_sb[:].unsqueeze(1)
                                .to_broadcast([128, TB, 128]),
                            op=OP.is_equal)
                        for g0 in range(0, TB, 4):
                            sz = min(4, TB - g0)
                            E1 = sz * 128
                            o1 = g0 * 128
                            b_et = pp.tile([128, 512], f32, tag="b_et")
                            for j in range(sz):
                                nc.tensor.matmul(
                                    out=b_et[:, j * 128:(j + 1) * 128],
                                    lhsT=xl_g[:, (g0 + j) * F:(g0 + j + 1) * F],
                                    rhs=id_sb[:], is_transpose=True,
                                    start=(j == 0), stop=False,
                                    skip_group_check=True)
                            nc.tensor.matmul(
                                out=b_et[:, 0:E1],
                                lhsT=xr_hi[:, b * 128:(b + 1) * 128],
                                rhs=m2[:, o1:o1 + E1],
                                start=False, stop=False, skip_group_check=True)
                            nc.tensor.matmul(
                                out=b_et[:, 0:E1],
                                lhsT=xr_lo[:, b * 128:(b + 1) * 128],
                                rhs=m2[:, o1:o1 + E1],
                                start=False, stop=True, skip_group_check=True)
                            lrel = gp.tile([128, 512], f32, tag="lrel")
                            if USE_PRELU:
                                nc.scalar.activation(
                                    out=lrel[:, 0:E1], in_=b_et[:, 0:E1],
                                    func=AF.Prelu, alpha=NEG)
                            else:
                                relu = gp.tile([128, 512], f32, tag="relu")
                                nc.scalar.activation(
                                    out=relu[:, 0:E1], in_=b_et[:, 0:E1],
                                    func=AF.Relu, scale=1.0 - NEG)
                                nc.vector.scalar_tensor_tensor(
                                    out=lrel[:, 0:E1], in0=b_et[:, 0:E1],
                                    scalar=NEG, in1=relu[:, 0:E1],
                                    op0=OP.mult, op1=OP.add)
                            b_s = pp.tile([128, 4 * H], f32, tag="b_s")
                            for j in range(sz):
                                nc.tensor.matmul(
                                    out=b_s[:, j * H:(j + 1) * H],
                                    lhsT=lrel[:, j * 128:(j + 1) * 128],
                                    rhs=att_sb[:, 0:H],
                                    start=(j == 0), stop=(j == sz - 1),
                                    skip_group_check=True)
                            w_exp = gp.tile([128, 4 * H], f32, tag="w_exp")
                            nc.scalar.activation(
                                out=w_exp[:, 0:sz * H], in_=b_s[:, 0:sz * H],
                                func=AF.Exp)
                            w_sb = gp.tile([128, 4 * (H + F)], bf16, tag="w_sb")
                            nc.vector.tensor_scalar_mul(
                                w_sb[:].rearrange("p (t x) -> p t x",
                                                  t=4)[:, 0:sz, 0:H],
                                w_exp[:, 0:sz * H]
                                    .rearrange("p (t h) -> p t h", t=sz),
                                1.0)
                            nc.vector.tensor_tensor(
                                out=w_sb[:].rearrange("p (t x) -> p t x",
                                                      t=4)[:, 0:sz, H:H + F]
                                    .rearrange("p t (h c) -> p t h c", h=H),
                                in0=xl_g[:, g0 * F:(g0 + sz) * F]
                                    .rearrange("p (t h c) -> p t h c",
                                               t=sz, h=H),
                                in1=w_exp[:, 0:sz * H]
                                    .rearrange("p (t h) -> p t h", t=sz)
                                    .unsqueeze(3)
                                    .to_broadcast([128, sz, H, F // H]),
                                op=OP.mult)
                            for j in range(sz):
                                nc.tensor.matmul(
                                    out=b_acc[:],
                                    lhsT=m[:, (g0 + j) * 128:(g0 + j + 1) * 128],
                                    rhs=w_sb[:, j * (H + F):(j + 1) * (H + F)],
                                    start=False,
                                    stop=(g0 + sz >= TB and j == sz - 1),
                                    skip_group_check=True)
                        ct += TB
                        # epilogue: divide by denominators, add bias
                        recip = op.tile([128, H], f32, tag="recip")
                        nc.vector.reciprocal(recip[:], b_acc[:, 0:H])
                        outb = op.tile([128, F], f32, tag="outb")
                        C = F // H
                        for h in range(H):
                            nc.vector.tensor_scalar_mul(
                                outb[:, h * C:(h + 1) * C],
                                b_acc[:, H + h * C:H + (h + 1) * C],
                                recip[:, h:h + 1])
                        nc.vector.tensor_tensor(out=outb[:], in0=outb[:],
                                                in1=bias_sb[:], op=OP.add)
                        epilogue(b, outb, op, pp1)

            def epi1(b, outb, wp, pp1):
                ps_h = pp1.tile([128, 128], f32, tag="ps_h")
                nc.tensor.matmul(out=ps_h[:], lhsT=outb[:], rhs=id_sb[:],
                                 is_transpose=True, start=True, stop=True)
                hT = wp.tile([128, 128], f32, tag="hT")
                nc.scalar.copy(hT[:], ps_h[:])
                ps_x = pp1.tile([128, 128], f32, tag="ps_x2")
                nc.tensor.matmul(out=ps_x[:], lhsT=hT[:], rhs=Wr2_sb[:],
                                 start=True, stop=True)
                nc.scalar.copy(xr2h_sb[:, b * 128:(b + 1) * 128], ps_x[:])
                nc.vector.tensor_tensor(
                    out=xr2l_sb[:, b * 128:(b + 1) * 128], in0=ps_x[:],
                    in1=xr2h_sb[:, b * 128:(b + 1) * 128], op=OP.subtract)
                ps_l = pp1.tile([128, 128], f32, tag="ps_x2")
                nc.tensor.matmul(out=ps_l[:], lhsT=hT[:], rhs=Wl2_sb[:],
                                 start=True, stop=True)
                l2s = wp.tile([128, 128], f32, tag="l2s")
                nc.scalar.copy(l2s[:], ps_l[:])
                nc.sync.dma_start(out=d_xl2loc[b * 128:(b + 1) * 128, :],
                                  in_=l2s[:])

            edge_layer(H1, d_xl1lo[:, :], d_xl1hi[:, :], xr1h_sb, xr1l_sb,
                       att1_sb, e01_sb, b1_sb, epi1)

            # ------------- phase C: AllGather layer-2 table -------------
            if profile_nocc:
                for s in range(NC):
                    nc.sync.dma_start(
                        out=d_xl2sh[s * NLOCP:(s + 1) * NLOCP, :],
                        in_=d_xl2loc[:, :])
            else:
                nc.gpsimd.collective_compute(
                    "AllGather", mybir.AluOpType.bypass,
                    replica_groups=[list(range(NC))],
                    ins=[d_xl2loc[:, :]], outs=[d_xl2sh[:, :]],
                )
            if GATHER_FROM_SHARED:
                d_l2lo, d_l2hi = d_xl2sh[0:HALF, :], d_xl2sh[HALF:NPAD, :]
            else:
                nc.sync.dma_start(out=d_xl2lo[:, :], in_=d_xl2sh[0:HALF, :])
                nc.scalar.dma_start(out=d_xl2hi[:, :],
                                    in_=d_xl2sh[HALF:NPAD, :])
                d_l2lo, d_l2hi = d_xl2lo[:, :], d_xl2hi[:, :]

            # ---------------- phase E: layer-2 edges ----------------
            def epi2(b, outb, wp, pp1):
                nc.sync.dma_start(out=d_out[b * 128:(b + 1) * 128, :],
                                  in_=outb[:])

            edge_layer(1, d_l2lo, d_l2hi, xr2h_sb, xr2l_sb, att2_sb,
                       e02_sb, b2_sb, epi2)

    nc.compile()
    return nc


# --------------------------------------------------------------------------
# Entry point
# --------------------------------------------------------------------------

_NC_CACHE = {}


def kernel(x, edge_index, edge_attr, Wl1, Wr1, att1, b1, Wl2, Wr2, att2, b2,
           cfg=None, _want_results=False):
    from concourse.bass_utils import run_bass_kernel_spmd

    cfg = cfg or CFG
    in_maps, meta = host_prep(x, edge_index, Wl1, Wr1, att1, b1,
                              Wl2, Wr2, att2, b2, cfg)
    key = (cfg.N, cfg.E, tuple(meta["pcnt"].tolist()))
    nc = _NC_CACHE.get(key)
    if nc is None:
        nc = build_nc(cfg, meta)
        _NC_CACHE[key] = nc
    res = run_bass_kernel_spmd(nc, in_maps, core_ids=list(range(cfg.ncores)))
    out = np.empty((cfg.N, cfg.F), dtype=np.float32)
    for c in range(cfg.ncores):
        out[c * cfg.nloc:(c + 1) * cfg.nloc] = \
            res.results[c]["outloc"][:cfg.nloc]
    if _want_results:
        return out, res
    return out


# revision 16
# speedup vs baseline: 1.5470x; 1.3512x over previous
"""GATv2 (2-layer, N=50000, E=800000) on 8 Trainium2 NeuronCores.

Strategy (self-contained; shapes hardcoded for nn_GATUnit_34067680592302):
  - Nodes partitioned across 8 cores (6250 each, padded to 6272 = 49 blocks
    of 128). Edges (incl. self-loops) assigned by destination node and sorted
    by destination, so scatter-softmax / segment-sum stay core-local.
  - Per layer, every core holds the full "source transform" table
    xl = x @ Wl in its DRAM (layer 1: computed locally from replicated x;
    layer 2: local h shard transformed then AllGather'ed), and bulk-gathers
    xl[src] rows per destination block with ONE dma_gather per block-half
    (int16 indices limit a gather table to 32K rows, so the table is split
    in two 25088-row halves and each block's edge list is stored as
    [half0-edges | pad | half1-edges | pad], each padded to a 128 multiple
    with index-0 rows that are masked out via dst=-1).
  - Per 128-node block, attention + weighted aggregation accumulate in PSUM
    via selection-matrix matmuls; a final reciprocal-scale epilogue divides
    by the softmax denominators (max-subtraction is skipped: |s| <= ~10 so
    exp() is safe in fp32).
"""
import sys
sys.path.insert(0, "/opt/trn_rl_repo")

import numpy as np

NEG = 0.2
USE_PRELU = True  # HW Prelu == leaky_relu(x, alpha); CoreSim lacks it
GATHER_FROM_SHARED = True  # layer-2 gathers read the AllGather buf directly
BCAST_VIA_DMA = True  # dstrep via sync-DMA broadcast (else gpsimd ucode)


class Cfg:
    def __init__(self, N=50000, E=800000, ncores=8, nloc=6250, F=128):
        assert N == ncores * nloc
        self.N, self.E, self.ncores, self.nloc, self.F = N, E, ncores, nloc, F
        self.nblk = -(-nloc // 128)          # blocks of 128 nodes per core
        self.nlocp = self.nblk * 128         # padded local nodes
        self.npad = ncores * self.nlocp      # padded global nodes
        self.half = self.npad // 2           # gather-table half size (<32K)
        assert self.npad % 1024 == 0 and self.half < 32768


CFG = Cfg()


# --------------------------------------------------------------------------
# Host-side preprocessing
# --------------------------------------------------------------------------

def host_prep(x, edge_index, Wl1, Wr1, att1, b1, Wl2, Wr2, att2, b2, cfg):
    N, E, NC, NLOC = cfg.N, cfg.E, cfg.ncores, cfg.nloc
    NBLK, NLOCP, NPAD, F = cfg.nblk, cfg.nlocp, cfg.npad, cfg.F
    HALF = cfg.half

    src0 = np.asarray(edge_index[0]).astype(np.int64)
    dst0 = np.asarray(edge_index[1]).astype(np.int64)
    loops = np.arange(N, dtype=np.int64)
    SRC = np.concatenate([src0, loops])
    DST = np.concatenate([dst0, loops])
    shard = DST // NLOC
    src_g = ((SRC // NLOC) * NLOCP + (SRC % NLOC)).astype(np.int32)
    dst_loc = (DST - shard * NLOC).astype(np.int64)

    per_core = []
    n_lo = np.zeros((NC, NBLK), dtype=np.int64)
    n_hi = np.zeros((NC, NBLK), dtype=np.int64)
    for c in range(NC):
        sel = shard == c
        sg, dl = src_g[sel], dst_loc[sel]
        order = np.argsort(dl, kind="stable")
        sg, dl = sg[order], dl[order]
        blk = dl // 128
        lo = sg < HALF
        per_core.append((sg, dl, blk, lo))
        n_lo[c] = np.bincount(blk[lo], minlength=NBLK)
        n_hi[c] = np.bincount(blk[~lo], minlength=NBLK)

    t0_b = (-(-n_lo.max(axis=0) // 128)).astype(np.int64)  # lo tiles per blk
    t1_b = (-(-n_hi.max(axis=0) // 128)).astype(np.int64)  # hi tiles per blk
    tiles_b = t0_b + t1_b
    pcnt = tiles_b * 128
    offs = np.concatenate([[0], np.cumsum(pcnt)])
    NTOT = int(offs[-1])
    NT = NTOT // 128
    PCMAX = int(pcnt.max())

    import ml_dtypes
    bf16 = ml_dtypes.bfloat16

    core_arrays = []
    for c in range(NC):
        sg, dl, blk, lo = per_core[c]
        idx_arr = np.zeros(NTOT, dtype=np.int16)
        dst_arr = np.full(NTOT, -1.0, dtype=np.float32)
        for b in range(NBLK):
            selb = blk == b
            sgb, dlb, lob = sg[selb], dl[selb] - 128 * b, lo[selb]
            o = int(offs[b])
            sl, dll = sgb[lob], dlb[lob]
            idx_arr[o:o + len(sl)] = sl.astype(np.int16)
            dst_arr[o:o + len(sl)] = dll
            o2 = o + int(t0_b[b]) * 128
            sh, dlh = sgb[~lob] - HALF, dlb[~lob]
            idx_arr[o2:o2 + len(sh)] = sh.astype(np.int16)
            dst_arr[o2:o2 + len(sh)] = dlh
        # dma_gather wraps index j to [j % 16, j // 16]; replicate x8 rows
        idx16 = np.tile(
            np.ascontiguousarray(idx_arr.reshape(-1, 16).T), (8, 1))
        dstcol = np.ascontiguousarray(
            dst_arr.reshape(NT, 128).T).astype(bf16)
        dstrow = np.full((NBLK, PCMAX), -1.0, dtype=np.float32)
        for b in range(NBLK):
            o = int(offs[b])
            dstrow[b, :int(pcnt[b])] = dst_arr[o:o + int(pcnt[b])]
        core_arrays.append(dict(idx16=idx16, dstcol=dstcol,
                                dstrow=dstrow.astype(bf16)))

    # node features, transposed + padded: xT[f, g] with g = s*NLOCP + j
    x = np.asarray(x, dtype=np.float32)
    xpad = np.zeros((NPAD, F), dtype=np.float32)
    for s in range(NC):
        xpad[s * NLOCP:s * NLOCP + NLOC] = x[s * NLOC:(s + 1) * NLOC]
    xTfull = np.ascontiguousarray(xpad.T)

    H1 = att1.shape[0]
    C1 = att1.shape[1]
    att1m = np.zeros((128, H1), dtype=np.float32)
    for h in range(H1):
        att1m[h * C1:(h + 1) * C1, h] = att1[h]
    att2m = np.zeros((128, 1), dtype=np.float32)
    att2m[:att2.shape[1], 0] = att2[0]

    iota = np.arange(128, dtype=np.float32)
    padmat = (np.arange(NLOCP) >= NLOC).astype(np.float32).reshape(1, NLOCP)

    shared = dict(
        xTfull=xTfull,
        Wl1=np.asarray(Wl1, np.float32), Wr1=np.asarray(Wr1, np.float32),
        Wl2=np.asarray(Wl2, np.float32), Wr2=np.asarray(Wr2, np.float32),
        att1m=att1m, att2m=att2m,
        bias1r=np.tile(np.asarray(b1, np.float32), (128, 1)),
        bias2r=np.tile(np.asarray(b2, np.float32), (128, 1)),
        id128=np.eye(128, dtype=np.float32),
        iorowb=np.tile(iota, (128, 1)).astype(bf16),
        iocol=iota.reshape(128, 1).copy(),
        iocolb=iota.reshape(128, 1).astype(bf16),
        padmat=padmat,
        e01=np.concatenate([np.ones(H1, np.float32),
                            np.zeros(F, np.float32)]).reshape(1, H1 + F),
        e02=np.concatenate([np.ones(1, np.float32),
                            np.zeros(F, np.float32)]).reshape(1, 1 + F),
    )
    in_maps = []
    for c in range(NC):
        m = dict(shared)
        m["xTloc"] = np.ascontiguousarray(xTfull[:, c * NLOCP:(c + 1) * NLOCP])
        m.update(core_arrays[c])
        in_maps.append(m)
    meta = dict(pcnt=pcnt, tiles_b=tiles_b, t0_b=t0_b, t1_b=t1_b,
                NT=NT, H1=H1, pcmax=PCMAX)
    return in_maps, meta


# --------------------------------------------------------------------------
# Device program
# --------------------------------------------------------------------------

def build_nc(cfg, meta, profile_nocc=False):
    import concourse.bacc as bacc
    import concourse.tile as tile
    from concourse import mybir

    f32 = mybir.dt.float32
    bf16 = mybir.dt.bfloat16
    i16 = mybir.dt.int16
    AF = mybir.ActivationFunctionType
    OP = mybir.AluOpType

    NC, F = cfg.ncores, cfg.F
    NBLK, NLOCP, NPAD, HALF = cfg.nblk, cfg.nlocp, cfg.npad, cfg.half
    NT, H1 = meta["NT"], meta["H1"]
    tiles_b, t0_b, t1_b = meta["tiles_b"], meta["t0_b"], meta["t1_b"]
    PCMAX = meta["pcmax"]
    TBMAX = PCMAX // 128

    nc = bacc.Bacc("TRN2", target_bir_lowering=False, num_swdge_queues=4)

    din = {}
    def ein(name, shape, dt=f32):
        din[name] = nc.dram_tensor(name, shape, dt, kind="ExternalInput")
        return din[name]

    d_xTfull = ein("xTfull", [128, NPAD])
    d_xTloc = ein("xTloc", [128, NLOCP])
    d_Wl1, d_Wr1 = ein("Wl1", [128, 128]), ein("Wr1", [128, 128])
    d_Wl2, d_Wr2 = ein("Wl2", [128, 128]), ein("Wr2", [128, 128])
    d_att1, d_att2 = ein("att1m", [128, H1]), ein("att2m", [128, 1])
    d_b1r, d_b2r = ein("bias1r", [128, F]), ein("bias2r", [128, F])
    d_id = ein("id128", [128, 128])
    d_iorowb = ein("iorowb", [128, 128], bf16)
    d_iocol = ein("iocol", [128, 1])
    d_iocolb = ein("iocolb", [128, 1], bf16)
    d_padm = ein("padmat", [1, NLOCP])
    d_e01, d_e02 = ein("e01", [1, H1 + F]), ein("e02", [1, 1 + F])
    d_idx16 = ein("idx16", [128, NT * 8], i16)
    d_dstcol = ein("dstcol", [128, NT], bf16)
    d_dstrow = ein("dstrow", [NBLK, PCMAX], bf16)

    d_out = nc.dram_tensor("outloc", [NLOCP, F], f32, kind="ExternalOutput")

    d_xl1lo = nc.dram_tensor("xl1lo", [HALF, F], f32)
    d_xl1hi = nc.dram_tensor("xl1hi", [HALF, F], f32)
    d_xl2sh = nc.dram_tensor("xl2sh", [NPAD, F], f32, addr_space="Shared")
    d_xl2loc = nc.dram_tensor("xl2loc", [NLOCP, F], f32)
    if not GATHER_FROM_SHARED:
        d_xl2lo = nc.dram_tensor("xl2lo", [HALF, F], f32)
        d_xl2hi = nc.dram_tensor("xl2hi", [HALF, F], f32)

    with tile.TileContext(nc) as tc:
        with tc.tile_pool(name="const", bufs=1) as cp:
            Wl1_sb = cp.tile_from(d_Wl1[:, :])
            Wr1_sb = cp.tile_from(d_Wr1[:, :])
            Wl2_sb = cp.tile_from(d_Wl2[:, :])
            Wr2_sb = cp.tile_from(d_Wr2[:, :])
            att1_sb = cp.tile_from(d_att1[:, :])
            att2_sb = cp.tile_from(d_att2[:, :])
            b1_sb = cp.tile_from(d_b1r[:, :])
            b2_sb = cp.tile_from(d_b2r[:, :])
            id_sb = cp.tile_from(d_id[:, :])
            iorowb_sb = cp.tile_from(d_iorowb[:, :])
            iocol_sb = cp.tile_from(d_iocol[:, :])
            iocolb_sb = cp.tile_from(d_iocolb[:, :])
            padm_sb = cp.tile_from(d_padm[:, :])
            e01_sb = cp.tile_from(d_e01[:, :])
            e02_sb = cp.tile_from(d_e02[:, :])
            idx_sb = cp.tile_from(d_idx16[:, :])
            dstc_sb = cp.tile_from(d_dstcol[:, :])
            xr1h_sb = cp.tile([128, NLOCP], bf16)
            xr1l_sb = cp.tile([128, NLOCP], bf16)
            xr2h_sb = cp.tile([128, NLOCP], bf16)
            xr2l_sb = cp.tile([128, NLOCP], bf16)

            # ---------------- phase A: layer-1 tables ----------------
            with (
                tc.tile_pool(name="tabs", bufs=4) as tp,
                tc.tile_pool(name="tabp", bufs=2, space="PSUM") as tpp,
            ):
                for t4 in range(-(-NBLK // 4)):  # local-shard xr1
                    q = min(4, NBLK - 4 * t4)
                    xt = tp.tile([128, 512], f32, tag="xt")
                    nc.scalar.dma_start(
                        out=xt[:, 0:q * 128],
                        in_=d_xTloc[:, t4 * 512:t4 * 512 + q * 128])
                    for j in range(q):
                        t = 4 * t4 + j
                        ps = tpp.tile([128, 128], f32, tag="psx")
                        nc.tensor.matmul(out=ps[:],
                                         lhsT=xt[:, j * 128:(j + 1) * 128],
                                         rhs=Wr1_sb[:], start=True, stop=True)
                        nc.scalar.copy(xr1h_sb[:, t * 128:(t + 1) * 128], ps[:])
                        nc.vector.tensor_tensor(
                            out=xr1l_sb[:, t * 128:(t + 1) * 128], in0=ps[:],
                            in1=xr1h_sb[:, t * 128:(t + 1) * 128],
                            op=OP.subtract)
                for t4 in range(NPAD // 512):  # full xl1 table locally
                    ps4 = tpp.tile([128, 512], f32, tag="ps4")
                    xt = tp.tile([128, 512], f32, tag="xt")
                    nc.scalar.dma_start(out=xt[:],
                                        in_=d_xTfull[:, t4 * 512:(t4 + 1) * 512])
                    for j in range(4):
                        nc.tensor.matmul(out=ps4[:, j * 128:(j + 1) * 128],
                                         lhsT=xt[:, j * 128:(j + 1) * 128],
                                         rhs=Wl1_sb[:],
                                         start=(j == 0), stop=(j == 3),
                                         skip_group_check=True)
                    stg = tp.tile([128, 512], f32, tag="stg")
                    nc.scalar.copy(stg[:], ps4[:])
                    half_t = NPAD // 1024  # 512-row chunks per half table
                    d_tab = d_xl1lo if t4 < half_t else d_xl1hi
                    r0 = (t4 if t4 < half_t else t4 - half_t) * 512
                    nc.sync.dma_start(
                        out=d_tab[r0:r0 + 512, :]
                            .rearrange("(t p) f -> p t f", p=128),
                        in_=stg[:].rearrange("p (t f) -> p t f", t=4),
                    )

            # ---------------- phase B: edge layers ----------------
            def edge_layer(H, d_lo, d_hi, xr_hi, xr_lo, att_sb, e0_sb,
                           bias_sb, epilogue):
                with (
                    tc.tile_pool(name="bp", bufs=2) as bp,
                    tc.tile_pool(name="gp", bufs=3) as gp,
                    tc.tile_pool(name="op", bufs=2) as op,
                    tc.tile_pool(name="epp", bufs=2, space="PSUM") as pp,
                    tc.tile_pool(name="epp1", bufs=1, space="PSUM") as pp1,
                ):
                    ct = 0
                    qrr = [0]
                    for b in range(NBLK):
                        T0, T1 = int(t0_b[b]), int(t1_b[b])
                        TB = T0 + T1
                        EB = TB * 128
                        b_acc = pp.tile([128, H + F], f32, tag="b_acc")
                        nc.tensor.matmul(out=b_acc[:],
                                         lhsT=padm_sb[0:1, b * 128:(b + 1) * 128],
                                         rhs=e0_sb[0:1, 0:H + F],
                                         start=True, stop=False,
                                         skip_group_check=True)
                        drst = bp.tile([1, PCMAX], bf16, tag="drst")
                        nc.scalar.dma_start(out=drst[0:1, 0:EB],
                                            in_=d_dstrow[b:b + 1, 0:EB])
                        # bulk gather of xl[src] for the whole block (one
                        # dma_gather per table half)
                        xl_g = bp.tile([128, TBMAX * F], f32, tag="xl_g")
                        # dma_gather breaks on HW above 1024 indices; use two
                        # balanced chunks per half, spread across the 4 SWDGE
                        # queues so transfers overlap
                        for tab, ta, tb in ((d_lo, 0, T0), (d_hi, T0, TB)):
                            nt = tb - ta
                            for c0, tn in ((ta, (nt + 1) // 2),
                                           (ta + (nt + 1) // 2, nt // 2)):
                                if tn == 0:
                                    continue
                                assert tn <= 8
                                nc.gpsimd.dma_gather(
                                    out_ap=xl_g[:, c0 * F:(c0 + tn) * F]
                                        .rearrange("p (t f) -> p t f", t=tn),
                                    in_ap=tab,
                                    idxs_ap=idx_sb[:, 8 * (ct + c0):
                                                   8 * (ct + c0 + tn)],
                                    num_idxs=tn * 128,
                                    num_idxs_reg=tn * 128,
                                    elem_size=F,
                                    queue_num=qrr[0] % 4,
                                )
                                qrr[0] += 1
                                # queue_num is rewritten post-scheduling to
                                # match the DMASW lane tile actually assigns
                        dstrep = bp.tile([128, PCMAX], bf16, tag="dstrep")
                        if BCAST_VIA_DMA:
                            nc.sync.dma_start(
                                out=dstrep[:, 0:EB],
                                in_=d_dstrow[b:b + 1, 0:EB]
                                    .to_broadcast([128, EB]))
                        else:
                            nc.gpsimd.partition_broadcast(
                                dstrep[:, 0:EB], drst[0:1, 0:EB])
                        m2 = bp.tile([128, PCMAX], bf16, tag="m2")
                        nc.vector.tensor_tensor(
                            out=m2[:, 0:EB], in0=dstrep[:, 0:EB],
                            in1=iocolb_sb[:].to_broadcast([128, EB]),
                            op=OP.is_equal)
                        m = bp.tile([128, PCMAX], bf16, tag="m")
                        nc.vector.tensor_tensor(
                            out=m[:, 0:EB].rearrange("p (t n) -> p t n", t=TB),
                            in0=dstc_sb[:, ct:ct + TB]
                                .unsqueeze(2).to_broadcast([128, TB, 128]),
                            in1=iorowb_sb[:].unsqueeze(1)
                                .to_broadcast([128, TB, 128]),
                            op=OP.is_equal)
                        for g0 in range(0, TB, 4):
                            sz = min(4, TB - g0)
                            E1 = sz * 128
                            o1 = g0 * 128
                            b_et = pp.tile([128, 512], f32, tag="b_et")
                            for j in range(sz):
                                nc.tensor.matmul(
                                    out=b_et[:, j * 128:(j + 1) * 128],
                                    lhsT=xl_g[:, (g0 + j) * F:(g0 + j + 1) * F],
                                    rhs=id_sb[:], is_transpose=True,
                                    start=(j == 0), stop=False,
                                    skip_group_check=True)
                            nc.tensor.matmul(
                                out=b_et[:, 0:E1],
                                lhsT=xr_hi[:, b * 128:(b + 1) * 128],
                                rhs=m2[:, o1:o1 + E1],
                                start=False, stop=False, skip_group_check=True)
                            nc.tensor.matmul(
                                out=b_et[:, 0:E1],
                                lhsT=xr_lo[:, b * 128:(b + 1) * 128],
                                rhs=m2[:, o1:o1 + E1],
                                start=False, stop=True, skip_group_check=True)
                            lrel = gp.tile([128, 512], f32, tag="lrel")
                            if USE_PRELU:
                                nc.scalar.activation(
                                    out=lrel[:, 0:E1], in_=b_et[:, 0:E1],
                                    func=AF.Prelu, alpha=NEG)
                            else:
                                relu = gp.tile([128, 512], f32, tag="relu")
                                nc.scalar.activation(
                                    out=relu[:, 0:E1], in_=b_et[:, 0:E1],
                                    func=AF.Relu, scale=1.0 - NEG)
                                nc.vector.scalar_tensor_tensor(
                                    out=lrel[:, 0:E1], in0=b_et[:, 0:E1],
                                    scalar=NEG, in1=relu[:, 0:E1],
                                    op0=OP.mult, op1=OP.add)
                            b_s = pp.tile([128, 4 * H], f32, tag="b_s")
                            for j in range(sz):
                                nc.tensor.matmul(
                                    out=b_s[:, j * H:(j + 1) * H],
                                    lhsT=lrel[:, j * 128:(j + 1) * 128],
                                    rhs=att_sb[:, 0:H],
                                    start=(j == 0), stop=(j == sz - 1),
                                    skip_group_check=True)
                            w_sb = gp.tile([128, 4 * (H + F)], bf16, tag="w_sb")
                            nc.scalar.activation(
                                out=w_sb[:].rearrange("p (t x) -> p t x",
                                                      t=4)[:, 0:sz, 0:H],
                                in_=b_s[:, 0:sz * H]
                                    .rearrange("p (t h) -> p t h", t=sz),
                                func=AF.Exp)
                            nc.vector.tensor_tensor(
                                out=w_sb[:].rearrange("p (t x) -> p t x",
                                                      t=4)[:, 0:sz, H:H + F]
                                    .rearrange("p t (h c) -> p t h c", h=H),
                                in0=xl_g[:, g0 * F:(g0 + sz) * F]
                                    .rearrange("p (t h c) -> p t h c",
                                               t=sz, h=H),
                                in1=w_sb[:].rearrange("p (t x) -> p t x",
                                                      t=4)[:, 0:sz, 0:H]
                                    .unsqueeze(3)
                                    .to_broadcast([128, sz, H, F // H]),
                                op=OP.mult)
                            for j in range(sz):
                                nc.tensor.matmul(
                                    out=b_acc[:],
                                    lhsT=m[:, (g0 + j) * 128:(g0 + j + 1) * 128],
                                    rhs=w_sb[:, j * (H + F):(j + 1) * (H + F)],
                                    start=False,
                                    stop=(g0 + sz >= TB and j == sz - 1),
                                    skip_group_check=True)
                        ct += TB
                        # epilogue: divide by denominators, add bias
                        recip = op.tile([128, H], f32, tag="recip")
                        nc.vector.reciprocal(recip[:], b_acc[:, 0:H])
                        outb = op.tile([128, F], f32, tag="outb")
                        C = F // H
                        for h in range(H):
                            nc.vector.tensor_scalar_mul(
                                outb[:, h * C:(h + 1) * C],
                                b_acc[:, H + h * C:H + (h + 1) * C],
                                recip[:, h:h + 1])
                        nc.vector.tensor_tensor(out=outb[:], in0=outb[:],
                                                in1=bias_sb[:], op=OP.add)
                        epilogue(b, outb, op, pp1)

            def epi1(b, outb, wp, pp1):
                ps_h = pp1.tile([128, 128], f32, tag="ps_h")
                nc.tensor.matmul(out=ps_h[:], lhsT=outb[:], rhs=id_sb[:],
                                 is_transpose=True, start=True, stop=True)
                hT = wp.tile([128, 128], f32, tag="hT")
                nc.scalar.copy(hT[:], ps_h[:])
                ps_x = pp1.tile([128, 128], f32, tag="ps_x2")
                nc.tensor.matmul(out=ps_x[:], lhsT=hT[:], rhs=Wr2_sb[:],
                                 start=True, stop=True)
                nc.scalar.copy(xr2h_sb[:, b * 128:(b + 1) * 128], ps_x[:])
                nc.vector.tensor_tensor(
                    out=xr2l_sb[:, b * 128:(b + 1) * 128], in0=ps_x[:],
                    in1=xr2h_sb[:, b * 128:(b + 1) * 128], op=OP.subtract)
                ps_l = pp1.tile([128, 128], f32, tag="ps_x2")
                nc.tensor.matmul(out=ps_l[:], lhsT=hT[:], rhs=Wl2_sb[:],
                                 start=True, stop=True)
                l2s = wp.tile([128, 128], f32, tag="l2s")
                nc.scalar.copy(l2s[:], ps_l[:])
                nc.sync.dma_start(out=d_xl2loc[b * 128:(b + 1) * 128, :],
                                  in_=l2s[:])

            edge_layer(H1, d_xl1lo[:, :], d_xl1hi[:, :], xr1h_sb, xr1l_sb,
                       att1_sb, e01_sb, b1_sb, epi1)

            # ------------- phase C: AllGather layer-2 table -------------
            if profile_nocc:
                for s in range(NC):
                    nc.sync.dma_start(
                        out=d_xl2sh[s * NLOCP:(s + 1) * NLOCP, :],
                        in_=d_xl2loc[:, :])
            else:
                nc.gpsimd.collective_compute(
                    "AllGather", mybir.AluOpType.bypass,
                    replica_groups=[list(range(NC))],
                    ins=[d_xl2loc[:, :]], outs=[d_xl2sh[:, :]],
                )
            if GATHER_FROM_SHARED:
                d_l2lo, d_l2hi = d_xl2sh[0:HALF, :], d_xl2sh[HALF:NPAD, :]
            else:
                nc.sync.dma_start(out=d_xl2lo[:, :], in_=d_xl2sh[0:HALF, :])
                nc.scalar.dma_start(out=d_xl2hi[:, :],
                                    in_=d_xl2sh[HALF:NPAD, :])
                d_l2lo, d_l2hi = d_xl2lo[:, :], d_xl2hi[:, :]

            # ---------------- phase E: layer-2 edges ----------------
            def epi2(b, outb, wp, pp1):
                nc.sync.dma_start(out=d_out[b * 128:(b + 1) * 128, :],
                                  in_=outb[:])

            edge_layer(1, d_l2lo, d_l2hi, xr2h_sb, xr2l_sb, att2_sb,
                       e02_sb, b2_sb, epi2)

    # The SWDGE ucode locks each semaphore to one queue, but tile's DMASW
    # lane rotation follows scheduled order (which can differ from emission
    # order). Re-derive queue_num from the lane tile actually assigned so
    # lane <-> queue is a pure function (lane % 4).
    for f in nc.m.functions:
        for blk in f.blocks:
            for inst in blk.instructions:
                if type(inst).__name__ == "InstDMAGatherAnt":
                    si = inst.sync_info
                    for u in (si.on_update or []) if si else []:
                        nmu = str(getattr(u, "ant_name", "") or "")
                        if nmu.startswith("DMASW"):
                            inst.queue_num = int(nmu.split("_")[0][5:]) % 4

    nc.compile()
    return nc


# --------------------------------------------------------------------------
# Entry point
# --------------------------------------------------------------------------

_NC_CACHE = {}


def kernel(x, edge_index, edge_attr, Wl1, Wr1, att1, b1, Wl2, Wr2, att2, b2,
           cfg=None, _want_results=False):
    from concourse.bass_utils import run_bass_kernel_spmd

    cfg = cfg or CFG
    in_maps, meta = host_prep(x, edge_index, Wl1, Wr1, att1, b1,
                              Wl2, Wr2, att2, b2, cfg)
    key = (cfg.N, cfg.E, tuple(meta["pcnt"].tolist()))
    nc = _NC_CACHE.get(key)
    if nc is None:
        nc = build_nc(cfg, meta)
        _NC_CACHE[key] = nc
    res = run_bass_kernel_spmd(nc, in_maps, core_ids=list(range(cfg.ncores)))
    out = np.empty((cfg.N, cfg.F), dtype=np.float32)
    for c in range(cfg.ncores):
        out[c * cfg.nloc:(c + 1) * cfg.nloc] = \
            res.results[c]["outloc"][:cfg.nloc]
    if _want_results:
        return out, res
    return out


# revision 25
# speedup vs baseline: 1.7073x; 1.1036x over previous
"""GATv2 (2-layer, N=50000, E=800000) on 8 Trainium2 NeuronCores.

Strategy (self-contained; shapes hardcoded for nn_GATUnit_34067680592302):
  - Nodes partitioned across 8 cores (6250 each, padded to 6272 = 49 blocks
    of 128). Edges (incl. self-loops) assigned by destination node and sorted
    by destination, so scatter-softmax / segment-sum stay core-local.
  - Per layer, every core holds the full "source transform" table
    xl = x @ Wl in its DRAM (layer 1: computed locally from replicated x;
    layer 2: local h shard transformed then AllGather'ed), and bulk-gathers
    xl[src] rows per destination block with ONE dma_gather per block-half
    (int16 indices limit a gather table to 32K rows, so the table is split
    in two 25088-row halves and each block's edge list is stored as
    [half0-edges | pad | half1-edges | pad], each padded to a 128 multiple
    with index-0 rows that are masked out via dst=-1).
  - Per 128-node block, attention + weighted aggregation accumulate in PSUM
    via selection-matrix matmuls; a final reciprocal-scale epilogue divides
    by the softmax denominators (max-subtraction is skipped: |s| <= ~10 so
    exp() is safe in fp32).
"""
import sys
sys.path.insert(0, "/opt/trn_rl_repo")

import numpy as np

NEG = 0.2
USE_PRELU = True  # HW Prelu == leaky_relu(x, alpha); CoreSim lacks it
GATHER_FROM_SHARED = True  # layer-2 gathers read the AllGather buf directly
BCAST_VIA_DMA = True  # dstrep via sync-DMA broadcast (else gpsimd ucode)


class Cfg:
    def __init__(self, N=50000, E=800000, ncores=8, nloc=6250, F=128):
        assert N == ncores * nloc
        self.N, self.E, self.ncores, self.nloc, self.F = N, E, ncores, nloc, F
        self.nblk = -(-nloc // 128)          # blocks of 128 nodes per core
        self.nlocp = self.nblk * 128         # padded local nodes
        self.npad = ncores * self.nlocp      # padded global nodes
        self.half = self.npad // 2           # gather-table half size (<32K)
        assert self.npad % 1024 == 0 and self.half < 32768


CFG = Cfg()


# --------------------------------------------------------------------------
# Host-side preprocessing
# --------------------------------------------------------------------------

def host_prep(x, edge_index, Wl1, Wr1, att1, b1, Wl2, Wr2, att2, b2, cfg):
    N, E, NC, NLOC = cfg.N, cfg.E, cfg.ncores, cfg.nloc
    NBLK, NLOCP, NPAD, F = cfg.nblk, cfg.nlocp, cfg.npad, cfg.F
    HALF = cfg.half

    src0 = np.asarray(edge_index[0]).astype(np.int64)
    dst0 = np.asarray(edge_index[1]).astype(np.int64)
    loops = np.arange(N, dtype=np.int64)
    SRC = np.concatenate([src0, loops])
    DST = np.concatenate([dst0, loops])
    shard = DST // NLOC
    src_g = ((SRC // NLOC) * NLOCP + (SRC % NLOC)).astype(np.int32)
    dst_loc = (DST - shard * NLOC).astype(np.int64)

    per_core = []
    n_lo = np.zeros((NC, NBLK), dtype=np.int64)
    n_hi = np.zeros((NC, NBLK), dtype=np.int64)
    for c in range(NC):
        sel = shard == c
        sg, dl = src_g[sel], dst_loc[sel]
        order = np.argsort(dl, kind="stable")
        sg, dl = sg[order], dl[order]
        blk = dl // 128
        lo = sg < HALF
        per_core.append((sg, dl, blk, lo))
        n_lo[c] = np.bincount(blk[lo], minlength=NBLK)
        n_hi[c] = np.bincount(blk[~lo], minlength=NBLK)

    t0_b = (-(-n_lo.max(axis=0) // 128)).astype(np.int64)  # lo tiles per blk
    t1_b = (-(-n_hi.max(axis=0) // 128)).astype(np.int64)  # hi tiles per blk
    tiles_b = t0_b + t1_b
    pcnt = tiles_b * 128
    offs = np.concatenate([[0], np.cumsum(pcnt)])
    NTOT = int(offs[-1])
    NT = NTOT // 128
    PCMAX = int(pcnt.max())

    import ml_dtypes
    bf16 = ml_dtypes.bfloat16

    core_arrays = []
    for c in range(NC):
        sg, dl, blk, lo = per_core[c]
        idx_arr = np.zeros(NTOT, dtype=np.int16)
        dst_arr = np.full(NTOT, -1.0, dtype=np.float32)
        for b in range(NBLK):
            selb = blk == b
            sgb, dlb, lob = sg[selb], dl[selb] - 128 * b, lo[selb]
            o = int(offs[b])
            sl, dll = sgb[lob], dlb[lob]
            idx_arr[o:o + len(sl)] = sl.astype(np.int16)
            dst_arr[o:o + len(sl)] = dll
            o2 = o + int(t0_b[b]) * 128
            sh, dlh = sgb[~lob] - HALF, dlb[~lob]
            idx_arr[o2:o2 + len(sh)] = sh.astype(np.int16)
            dst_arr[o2:o2 + len(sh)] = dlh
        # dma_gather wraps index j to [j % 16, j // 16]; replicate x8 rows
        idx16 = np.tile(
            np.ascontiguousarray(idx_arr.reshape(-1, 16).T), (8, 1))
        dstcol = np.ascontiguousarray(
            dst_arr.reshape(NT, 128).T).astype(bf16)
        dstrow = np.full((NBLK, PCMAX), -1.0, dtype=np.float32)
        for b in range(NBLK):
            o = int(offs[b])
            dstrow[b, :int(pcnt[b])] = dst_arr[o:o + int(pcnt[b])]
        core_arrays.append(dict(idx16=idx16, dstcol=dstcol,
                                dstrow=dstrow.astype(bf16)))

    # node features, transposed + padded: xT[f, g] with g = s*NLOCP + j
    x = np.asarray(x, dtype=np.float32)
    xpad = np.zeros((NPAD, F), dtype=np.float32)
    for s in range(NC):
        xpad[s * NLOCP:s * NLOCP + NLOC] = x[s * NLOC:(s + 1) * NLOC]
    xTfull = np.ascontiguousarray(xpad.T)

    H1 = att1.shape[0]
    C1 = att1.shape[1]
    att1m = np.zeros((128, H1), dtype=np.float32)
    for h in range(H1):
        att1m[h * C1:(h + 1) * C1, h] = att1[h]
    att2m = np.zeros((128, 1), dtype=np.float32)
    att2m[:att2.shape[1], 0] = att2[0]

    iota = np.arange(128, dtype=np.float32)
    padmat = (np.arange(NLOCP) >= NLOC).astype(np.float32).reshape(1, NLOCP)

    shared = dict(
        xTfull=xTfull.astype(bf16),
        Wl1=np.asarray(Wl1, np.float32).astype(bf16),
        Wr1=np.asarray(Wr1, np.float32).astype(bf16),
        Wl2=np.asarray(Wl2, np.float32).astype(bf16),
        Wr2=np.asarray(Wr2, np.float32).astype(bf16),
        att1m=att1m.astype(bf16), att2m=att2m.astype(bf16),
        bias1r=np.tile(np.asarray(b1, np.float32), (128, 1)),
        bias2r=np.tile(np.asarray(b2, np.float32), (128, 1)),
        id128=np.eye(128, dtype=np.float32),
        iorowb=np.tile(iota, (128, 1)).astype(bf16),
        iocol=iota.reshape(128, 1).copy(),
        iocolb=iota.reshape(128, 1).astype(bf16),
        padmat=padmat,
        e01=np.concatenate([np.ones(H1, np.float32),
                            np.zeros(F, np.float32)]).reshape(1, H1 + F),
        e02=np.concatenate([np.ones(1, np.float32),
                            np.zeros(F, np.float32)]).reshape(1, 1 + F),
    )
    in_maps = []
    for c in range(NC):
        m = dict(shared)
        m["xTloc"] = np.ascontiguousarray(
            xTfull[:, c * NLOCP:(c + 1) * NLOCP]).astype(bf16)
        m.update(core_arrays[c])
        in_maps.append(m)
    meta = dict(pcnt=pcnt, tiles_b=tiles_b, t0_b=t0_b, t1_b=t1_b,
                NT=NT, H1=H1, pcmax=PCMAX)
    return in_maps, meta


# --------------------------------------------------------------------------
# Device program
# --------------------------------------------------------------------------

def build_nc(cfg, meta, profile_nocc=False):
    import concourse.bacc as bacc
    import concourse.tile as tile
    from concourse import mybir

    f32 = mybir.dt.float32
    bf16 = mybir.dt.bfloat16
    i16 = mybir.dt.int16
    AF = mybir.ActivationFunctionType
    OP = mybir.AluOpType

    NC, F = cfg.ncores, cfg.F
    NBLK, NLOCP, NPAD, HALF = cfg.nblk, cfg.nlocp, cfg.npad, cfg.half
    NT, H1 = meta["NT"], meta["H1"]
    tiles_b, t0_b, t1_b = meta["tiles_b"], meta["t0_b"], meta["t1_b"]
    PCMAX = meta["pcmax"]
    TBMAX = PCMAX // 128

    nc = bacc.Bacc("TRN2", target_bir_lowering=False, num_swdge_queues=4)

    din = {}
    def ein(name, shape, dt=f32):
        din[name] = nc.dram_tensor(name, shape, dt, kind="ExternalInput")
        return din[name]

    d_xTfull = ein("xTfull", [128, NPAD], bf16)
    d_xTloc = ein("xTloc", [128, NLOCP], bf16)
    d_Wl1, d_Wr1 = ein("Wl1", [128, 128], bf16), ein("Wr1", [128, 128], bf16)
    d_Wl2, d_Wr2 = ein("Wl2", [128, 128], bf16), ein("Wr2", [128, 128], bf16)
    d_att1 = ein("att1m", [128, H1], bf16)
    d_att2 = ein("att2m", [128, 1], bf16)
    d_b1r, d_b2r = ein("bias1r", [128, F]), ein("bias2r", [128, F])
    d_id = ein("id128", [128, 128])
    d_iorowb = ein("iorowb", [128, 128], bf16)
    d_iocol = ein("iocol", [128, 1])
    d_iocolb = ein("iocolb", [128, 1], bf16)
    d_padm = ein("padmat", [1, NLOCP])
    d_e01, d_e02 = ein("e01", [1, H1 + F]), ein("e02", [1, 1 + F])
    d_idx16 = ein("idx16", [128, NT * 8], i16)
    d_dstcol = ein("dstcol", [128, NT], bf16)
    d_dstrow = ein("dstrow", [NBLK, PCMAX], bf16)

    d_out = nc.dram_tensor("outloc", [NLOCP, F], f32, kind="ExternalOutput")

    d_xl1lo = nc.dram_tensor("xl1lo", [HALF, F], f32)
    d_xl1hi = nc.dram_tensor("xl1hi", [HALF, F], f32)
    d_xl2sh = nc.dram_tensor("xl2sh", [NPAD, F], f32, addr_space="Shared")
    d_xl2loc = nc.dram_tensor("xl2loc", [NLOCP, F], f32)
    if not GATHER_FROM_SHARED:
        d_xl2lo = nc.dram_tensor("xl2lo", [HALF, F], f32)
        d_xl2hi = nc.dram_tensor("xl2hi", [HALF, F], f32)

    with tile.TileContext(nc) as tc:
        with tc.tile_pool(name="const", bufs=1) as cp:
            Wl1_sb = cp.tile_from(d_Wl1[:, :])
            Wr1_sb = cp.tile_from(d_Wr1[:, :])
            Wl2_sb = cp.tile_from(d_Wl2[:, :])
            Wr2_sb = cp.tile_from(d_Wr2[:, :])
            att1_sb = cp.tile_from(d_att1[:, :])
            att2_sb = cp.tile_from(d_att2[:, :])
            b1_sb = cp.tile_from(d_b1r[:, :])
            b2_sb = cp.tile_from(d_b2r[:, :])
            id_sb = cp.tile_from(d_id[:, :])
            iorowb_sb = cp.tile_from(d_iorowb[:, :])
            iocol_sb = cp.tile_from(d_iocol[:, :])
            iocolb_sb = cp.tile_from(d_iocolb[:, :])
            padm_sb = cp.tile_from(d_padm[:, :])
            e01_sb = cp.tile_from(d_e01[:, :])
            e02_sb = cp.tile_from(d_e02[:, :])
            idx_sb = cp.tile_from(d_idx16[:, :])
            dstc_sb = cp.tile_from(d_dstcol[:, :])
            xr1h_sb = cp.tile([128, NLOCP], bf16)
            xr2h_sb = cp.tile([128, NLOCP], bf16)

            # ---------------- phase A: layer-1 tables ----------------
            with (
                tc.tile_pool(name="tabs", bufs=4) as tp,
                tc.tile_pool(name="tabp", bufs=2, space="PSUM") as tpp,
            ):
                for t4 in range(-(-NBLK // 4)):  # local-shard xr1
                    q = min(4, NBLK - 4 * t4)
                    xt = tp.tile([128, 512], bf16, tag="xt")
                    nc.scalar.dma_start(
                        out=xt[:, 0:q * 128],
                        in_=d_xTloc[:, t4 * 512:t4 * 512 + q * 128])
                    for j in range(q):
                        t = 4 * t4 + j
                        ps = tpp.tile([128, 128], f32, tag="psx")
                        nc.tensor.matmul(out=ps[:],
                                         lhsT=xt[:, j * 128:(j + 1) * 128],
                                         rhs=Wr1_sb[:], start=True, stop=True)
                        nc.scalar.copy(xr1h_sb[:, t * 128:(t + 1) * 128], ps[:])
                for t4 in range(NPAD // 512):  # full xl1 table locally
                    ps4 = tpp.tile([128, 512], f32, tag="ps4")
                    xt = tp.tile([128, 512], bf16, tag="xt")
                    nc.scalar.dma_start(out=xt[:],
                                        in_=d_xTfull[:, t4 * 512:(t4 + 1) * 512])
                    for j in range(4):
                        nc.tensor.matmul(out=ps4[:, j * 128:(j + 1) * 128],
                                         lhsT=xt[:, j * 128:(j + 1) * 128],
                                         rhs=Wl1_sb[:],
                                         start=(j == 0), stop=(j == 3),
                                         skip_group_check=True)
                    stg = tp.tile([128, 512], f32, tag="stg")
                    nc.scalar.copy(stg[:], ps4[:])
                    half_t = NPAD // 1024  # 512-row chunks per half table
                    d_tab = d_xl1lo if t4 < half_t else d_xl1hi
                    r0 = (t4 if t4 < half_t else t4 - half_t) * 512
                    nc.sync.dma_start(
                        out=d_tab[r0:r0 + 512, :]
                            .rearrange("(t p) f -> p t f", p=128),
                        in_=stg[:].rearrange("p (t f) -> p t f", t=4),
                    )

            # ---------------- phase B: edge layers ----------------
            def edge_layer(H, d_lo, d_hi, xr_hi, att_sb, e0_sb,
                           bias_sb, epilogue):
                with (
                    tc.tile_pool(name="bp", bufs=2) as bp,
                    tc.tile_pool(name="gp", bufs=3) as gp,
                    tc.tile_pool(name="op", bufs=2) as op,
                    tc.tile_pool(name="epp", bufs=2, space="PSUM") as pp,
                    tc.tile_pool(name="epp1", bufs=1, space="PSUM") as pp1,
                ):
                    ct = 0
                    qrr = [0]
                    for b in range(NBLK):
                        T0, T1 = int(t0_b[b]), int(t1_b[b])
                        TB = T0 + T1
                        EB = TB * 128
                        b_acc = pp.tile([128, H + F], f32, tag="b_acc")
                        nc.tensor.matmul(out=b_acc[:],
                                         lhsT=padm_sb[0:1, b * 128:(b + 1) * 128],
                                         rhs=e0_sb[0:1, 0:H + F],
                                         start=True, stop=False,
                                         skip_group_check=True)
                        drst = bp.tile([1, PCMAX], bf16, tag="drst")
                        nc.scalar.dma_start(out=drst[0:1, 0:EB],
                                            in_=d_dstrow[b:b + 1, 0:EB])
                        # bulk gather of xl[src] for the whole block (one
                        # dma_gather per table half)
                        xl_g = bp.tile([128, TBMAX * F], f32, tag="xl_g")
                        # dma_gather breaks on HW above 1024 indices; use two
                        # balanced chunks per half, spread across the 4 SWDGE
                        # queues so transfers overlap
                        for tab, ta, tb in ((d_lo, 0, T0), (d_hi, T0, TB)):
                            nt = tb - ta
                            for c0, tn in ((ta, (nt + 1) // 2),
                                           (ta + (nt + 1) // 2, nt // 2)):
                                if tn == 0:
                                    continue
                                assert tn <= 8
                                nc.gpsimd.dma_gather(
                                    out_ap=xl_g[:, c0 * F:(c0 + tn) * F]
                                        .rearrange("p (t f) -> p t f", t=tn),
                                    in_ap=tab,
                                    idxs_ap=idx_sb[:, 8 * (ct + c0):
                                                   8 * (ct + c0 + tn)],
                                    num_idxs=tn * 128,
                                    num_idxs_reg=tn * 128,
                                    elem_size=F,
                                    queue_num=qrr[0] % 4,
                                )
                                qrr[0] += 1
                                # queue_num is rewritten post-scheduling to
                                # match the DMASW lane tile actually assigns
                        dstrep = bp.tile([128, PCMAX], bf16, tag="dstrep")
                        if BCAST_VIA_DMA:
                            nc.sync.dma_start(
                                out=dstrep[:, 0:EB],
                                in_=d_dstrow[b:b + 1, 0:EB]
                                    .to_broadcast([128, EB]))
                        else:
                            nc.gpsimd.partition_broadcast(
                                dstrep[:, 0:EB], drst[0:1, 0:EB])
                        m2 = bp.tile([128, PCMAX], bf16, tag="m2")
                        nc.vector.tensor_tensor(
                            out=m2[:, 0:EB], in0=dstrep[:, 0:EB],
                            in1=iocolb_sb[:].to_broadcast([128, EB]),
                            op=OP.is_equal)
                        m = bp.tile([128, PCMAX], bf16, tag="m")
                        nc.vector.tensor_tensor(
                            out=m[:, 0:EB].rearrange("p (t n) -> p t n", t=TB),
                            in0=dstc_sb[:, ct:ct + TB]
                                .unsqueeze(2).to_broadcast([128, TB, 128]),
                            in1=iorowb_sb[:].unsqueeze(1)
                                .to_broadcast([128, TB, 128]),
                            op=OP.is_equal)
                        for g0 in range(0, TB, 4):
                            sz = min(4, TB - g0)
                            E1 = sz * 128
                            o1 = g0 * 128
                            b_et = pp.tile([128, 512], f32, tag="b_et")
                            for j in range(sz):
                                nc.tensor.matmul(
                                    out=b_et[:, j * 128:(j + 1) * 128],
                                    lhsT=xl_g[:, (g0 + j) * F:(g0 + j + 1) * F],
                                    rhs=id_sb[:], is_transpose=True,
                                    start=(j == 0), stop=False,
                                    skip_group_check=True)
                            nc.tensor.matmul(
                                out=b_et[:, 0:E1],
                                lhsT=xr_hi[:, b * 128:(b + 1) * 128],
                                rhs=m2[:, o1:o1 + E1],
                                start=False, stop=True, skip_group_check=True)
                            lrel = gp.tile([128, 512], bf16, tag="lrel")
                            if USE_PRELU:
                                nc.scalar.activation(
                                    out=lrel[:, 0:E1], in_=b_et[:, 0:E1],
                                    func=AF.Prelu, alpha=NEG)
                            else:
                                relu = gp.tile([128, 512], f32, tag="relu")
                                nc.scalar.activation(
                                    out=relu[:, 0:E1], in_=b_et[:, 0:E1],
                                    func=AF.Relu, scale=1.0 - NEG)
                                nc.vector.scalar_tensor_tensor(
                                    out=lrel[:, 0:E1], in0=b_et[:, 0:E1],
                                    scalar=NEG, in1=relu[:, 0:E1],
                                    op0=OP.mult, op1=OP.add)
                            b_s = pp.tile([128, 4 * H], f32, tag="b_s")
                            for j in range(sz):
                                nc.tensor.matmul(
                                    out=b_s[:, j * H:(j + 1) * H],
                                    lhsT=lrel[:, j * 128:(j + 1) * 128],
                                    rhs=att_sb[:, 0:H],
                                    start=(j == 0), stop=(j == sz - 1),
                                    skip_group_check=True)
                            w_sb = gp.tile([128, 4 * (H + F)], bf16, tag="w_sb")
                            nc.scalar.activation(
                                out=w_sb[:].rearrange("p (t x) -> p t x",
                                                      t=4)[:, 0:sz, 0:H],
                                in_=b_s[:, 0:sz * H]
                                    .rearrange("p (t h) -> p t h", t=sz),
                                func=AF.Exp)
                            nc.vector.tensor_tensor(
                                out=w_sb[:].rearrange("p (t x) -> p t x",
                                                      t=4)[:, 0:sz, H:H + F]
                                    .rearrange("p t (h c) -> p t h c", h=H),
                                in0=xl_g[:, g0 * F:(g0 + sz) * F]
                                    .rearrange("p (t h c) -> p t h c",
                                               t=sz, h=H),
                                in1=w_sb[:].rearrange("p (t x) -> p t x",
                                                      t=4)[:, 0:sz, 0:H]
                                    .unsqueeze(3)
                                    .to_broadcast([128, sz, H, F // H]),
                                op=OP.mult)
                            for j in range(sz):
                                nc.tensor.matmul(
                                    out=b_acc[:],
                                    lhsT=m[:, (g0 + j) * 128:(g0 + j + 1) * 128],
                                    rhs=w_sb[:, j * (H + F):(j + 1) * (H + F)],
                                    start=False,
                                    stop=(g0 + sz >= TB and j == sz - 1),
                                    skip_group_check=True)
                        ct += TB
                        # epilogue: divide by denominators, add bias
                        recip = op.tile([128, H], f32, tag="recip")
                        nc.vector.reciprocal(recip[:], b_acc[:, 0:H])
                        outb = op.tile([128, F], f32, tag="outb")
                        C = F // H
                        for h in range(H):
                            nc.vector.tensor_scalar_mul(
                                outb[:, h * C:(h + 1) * C],
                                b_acc[:, H + h * C:H + (h + 1) * C],
                                recip[:, h:h + 1])
                        nc.vector.tensor_tensor(out=outb[:], in0=outb[:],
                                                in1=bias_sb[:], op=OP.add)
                        epilogue(b, outb, op, pp1)

            def epi1(b, outb, wp, pp1):
                ps_h = pp1.tile([128, 128], f32, tag="ps_h")
                nc.tensor.matmul(out=ps_h[:], lhsT=outb[:], rhs=id_sb[:],
                                 is_transpose=True, start=True, stop=True)
                hT = wp.tile([128, 128], bf16, tag="hT")
                nc.scalar.copy(hT[:], ps_h[:])
                ps_x = pp1.tile([128, 128], f32, tag="ps_x2")
                nc.tensor.matmul(out=ps_x[:], lhsT=hT[:], rhs=Wr2_sb[:],
                                 start=True, stop=True)
                nc.scalar.copy(xr2h_sb[:, b * 128:(b + 1) * 128], ps_x[:])
                ps_l = pp1.tile([128, 128], f32, tag="ps_x2")
                nc.tensor.matmul(out=ps_l[:], lhsT=hT[:], rhs=Wl2_sb[:],
                                 start=True, stop=True)
                l2s = wp.tile([128, 128], f32, tag="l2s")
                nc.scalar.copy(l2s[:], ps_l[:])
                nc.sync.dma_start(out=d_xl2loc[b * 128:(b + 1) * 128, :],
                                  in_=l2s[:])

            edge_layer(H1, d_xl1lo[:, :], d_xl1hi[:, :], xr1h_sb,
                       att1_sb, e01_sb, b1_sb, epi1)

            # ------------- phase C: AllGather layer-2 table -------------
            if profile_nocc:
                for s in range(NC):
                    nc.sync.dma_start(
                        out=d_xl2sh[s * NLOCP:(s + 1) * NLOCP, :],
                        in_=d_xl2loc[:, :])
            else:
                nc.gpsimd.collective_compute(
                    "AllGather", mybir.AluOpType.bypass,
                    replica_groups=[list(range(NC))],
                    ins=[d_xl2loc[:, :]], outs=[d_xl2sh[:, :]],
                )
            if GATHER_FROM_SHARED:
                d_l2lo, d_l2hi = d_xl2sh[0:HALF, :], d_xl2sh[HALF:NPAD, :]
            else:
                nc.sync.dma_start(out=d_xl2lo[:, :], in_=d_xl2sh[0:HALF, :])
                nc.scalar.dma_start(out=d_xl2hi[:, :],
                                    in_=d_xl2sh[HALF:NPAD, :])
                d_l2lo, d_l2hi = d_xl2lo[:, :], d_xl2hi[:, :]

            # ---------------- phase E: layer-2 edges ----------------
            def epi2(b, outb, wp, pp1):
                nc.sync.dma_start(out=d_out[b * 128:(b + 1) * 128, :],
                                  in_=outb[:])

            edge_layer(1, d_l2lo, d_l2hi, xr2h_sb, att2_sb,
                       e02_sb, b2_sb, epi2)

    # The SWDGE ucode locks each semaphore to one queue, but tile's DMASW
    # lane rotation follows scheduled order (which can differ from emission
    # order). Re-derive queue_num from the lane tile actually assigned so
    # lane <-> queue is a pure function (lane % 4).
    for f in nc.m.functions:
        for blk in f.blocks:
            for inst in blk.instructions:
                if type(inst).__name__ == "InstDMAGatherAnt":
                    si = inst.sync_info
                    for u in (si.on_update or []) if si else []:
                        nmu = str(getattr(u, "ant_name", "") or "")
                        if nmu.startswith("DMASW"):
                            inst.queue_num = int(nmu.split("_")[0][5:]) % 4

    nc.compile()
    return nc


# --------------------------------------------------------------------------
# Entry point
# --------------------------------------------------------------------------

_NC_CACHE = {}


def kernel(x, edge_index, edge_attr, Wl1, Wr1, att1, b1, Wl2, Wr2, att2, b2,
           cfg=None, _want_results=False):
    from concourse.bass_utils import run_bass_kernel_spmd

    cfg = cfg or CFG
    in_maps, meta = host_prep(x, edge_index, Wl1, Wr1, att1, b1,
                              Wl2, Wr2, att2, b2, cfg)
    key = (cfg.N, cfg.E, tuple(meta["pcnt"].tolist()))
    nc = _NC_CACHE.get(key)
    if nc is None:
        nc = build_nc(cfg, meta)
        _NC_CACHE[key] = nc
    res = run_bass_kernel_spmd(nc, in_maps, core_ids=list(range(cfg.ncores)))
    out = np.empty((cfg.N, cfg.F), dtype=np.float32)
    for c in range(cfg.ncores):
        out[c * cfg.nloc:(c + 1) * cfg.nloc] = \
            res.results[c]["outloc"][:cfg.nloc]
    if _want_results:
        return out, res
    return out


# revision 27
# speedup vs baseline: 2.3499x; 1.3764x over previous
"""GATv2 (2-layer, N=50000, E=800000) on 8 Trainium2 NeuronCores.

Strategy (self-contained; shapes hardcoded for nn_GATUnit_34067680592302):
  - Nodes partitioned across 8 cores (6250 each, padded to 6272 = 49 blocks
    of 128). Edges (incl. self-loops) assigned by destination node and sorted
    by destination, so scatter-softmax / segment-sum stay core-local.
  - Per layer, every core holds the full "source transform" table
    xl = x @ Wl in its DRAM (layer 1: computed locally from replicated x;
    layer 2: local h shard transformed then AllGather'ed), and bulk-gathers
    xl[src] rows per destination block with ONE dma_gather per block-half
    (int16 indices limit a gather table to 32K rows, so the table is split
    in two 25088-row halves and each block's edge list is stored as
    [half0-edges | pad | half1-edges | pad], each padded to a 128 multiple
    with index-0 rows that are masked out via dst=-1).
  - Per 128-node block, attention + weighted aggregation accumulate in PSUM
    via selection-matrix matmuls; a final reciprocal-scale epilogue divides
    by the softmax denominators (max-subtraction is skipped: |s| <= ~10 so
    exp() is safe in fp32).
"""
import sys
sys.path.insert(0, "/opt/trn_rl_repo")

import numpy as np

NEG = 0.2
USE_PRELU = True  # HW Prelu == leaky_relu(x, alpha); CoreSim lacks it
GATHER_FROM_SHARED = True  # layer-2 gathers read the AllGather buf directly
BCAST_VIA_DMA = True  # dstrep via sync-DMA broadcast (else gpsimd ucode)


class Cfg:
    def __init__(self, N=50000, E=800000, ncores=8, nloc=6250, F=128):
        assert N == ncores * nloc
        self.N, self.E, self.ncores, self.nloc, self.F = N, E, ncores, nloc, F
        self.nblk = -(-nloc // 128)          # blocks of 128 nodes per core
        self.nlocp = self.nblk * 128         # padded local nodes
        self.npad = ncores * self.nlocp      # padded global nodes
        self.half = self.npad // 2           # gather-table half size (<32K)
        assert self.npad % 1024 == 0 and self.half < 32768


CFG = Cfg()


# --------------------------------------------------------------------------
# Host-side preprocessing
# --------------------------------------------------------------------------

def host_prep(x, edge_index, Wl1, Wr1, att1, b1, Wl2, Wr2, att2, b2, cfg):
    N, E, NC, NLOC = cfg.N, cfg.E, cfg.ncores, cfg.nloc
    NBLK, NLOCP, NPAD, F = cfg.nblk, cfg.nlocp, cfg.npad, cfg.F
    HALF = cfg.half

    src0 = np.asarray(edge_index[0]).astype(np.int64)
    dst0 = np.asarray(edge_index[1]).astype(np.int64)
    loops = np.arange(N, dtype=np.int64)
    SRC = np.concatenate([src0, loops])
    DST = np.concatenate([dst0, loops])
    shard = DST // NLOC
    src_g = ((SRC // NLOC) * NLOCP + (SRC % NLOC)).astype(np.int32)
    dst_loc = (DST - shard * NLOC).astype(np.int64)

    per_core = []
    n_lo = np.zeros((NC, NBLK), dtype=np.int64)
    n_hi = np.zeros((NC, NBLK), dtype=np.int64)
    for c in range(NC):
        sel = shard == c
        sg, dl = src_g[sel], dst_loc[sel]
        order = np.argsort(dl, kind="stable")
        sg, dl = sg[order], dl[order]
        blk = dl // 128
        lo = sg < HALF
        per_core.append((sg, dl, blk, lo))
        n_lo[c] = np.bincount(blk[lo], minlength=NBLK)
        n_hi[c] = np.bincount(blk[~lo], minlength=NBLK)

    t0_b = (-(-n_lo.max(axis=0) // 128)).astype(np.int64)  # lo tiles per blk
    t1_b = (-(-n_hi.max(axis=0) // 128)).astype(np.int64)  # hi tiles per blk
    tiles_b = t0_b + t1_b
    pcnt = tiles_b * 128
    offs = np.concatenate([[0], np.cumsum(pcnt)])
    NTOT = int(offs[-1])
    NT = NTOT // 128
    PCMAX = int(pcnt.max())

    import ml_dtypes
    bf16 = ml_dtypes.bfloat16

    core_arrays = []
    for c in range(NC):
        sg, dl, blk, lo = per_core[c]
        idx_arr = np.zeros(NTOT, dtype=np.int16)
        dst_arr = np.full(NTOT, -1.0, dtype=np.float32)
        for b in range(NBLK):
            selb = blk == b
            sgb, dlb, lob = sg[selb], dl[selb] - 128 * b, lo[selb]
            o = int(offs[b])
            sl, dll = sgb[lob], dlb[lob]
            idx_arr[o:o + len(sl)] = sl.astype(np.int16)
            dst_arr[o:o + len(sl)] = dll
            o2 = o + int(t0_b[b]) * 128
            sh, dlh = sgb[~lob] - HALF, dlb[~lob]
            idx_arr[o2:o2 + len(sh)] = sh.astype(np.int16)
            dst_arr[o2:o2 + len(sh)] = dlh
        # dma_gather wraps index j to [j % 16, j // 16]; replicate x8 rows
        idx16 = np.tile(
            np.ascontiguousarray(idx_arr.reshape(-1, 16).T), (8, 1))
        dstcol = np.ascontiguousarray(
            dst_arr.reshape(NT, 128).T).astype(bf16)
        dstrow = np.full((NBLK, PCMAX), -1.0, dtype=np.float32)
        for b in range(NBLK):
            o = int(offs[b])
            dstrow[b, :int(pcnt[b])] = dst_arr[o:o + int(pcnt[b])]
        core_arrays.append(dict(idx16=idx16, dstcol=dstcol,
                                dstrow=dstrow.astype(bf16)))

    # node features, transposed + padded: xT[f, g] with g = s*NLOCP + j
    x = np.asarray(x, dtype=np.float32)
    xpad = np.zeros((NPAD, F), dtype=np.float32)
    for s in range(NC):
        xpad[s * NLOCP:s * NLOCP + NLOC] = x[s * NLOC:(s + 1) * NLOC]
    xTfull = np.ascontiguousarray(xpad.T)

    H1 = att1.shape[0]
    C1 = att1.shape[1]
    att1m = np.zeros((128, H1), dtype=np.float32)
    for h in range(H1):
        att1m[h * C1:(h + 1) * C1, h] = att1[h]
    att2m = np.zeros((128, 1), dtype=np.float32)
    att2m[:att2.shape[1], 0] = att2[0]

    iota = np.arange(128, dtype=np.float32)
    padmat = (np.arange(NLOCP) >= NLOC).astype(np.float32).reshape(1, NLOCP)

    shared = dict(
        xTfull=xTfull.astype(bf16),
        Wl1=np.asarray(Wl1, np.float32).astype(bf16),
        Wr1=np.asarray(Wr1, np.float32).astype(bf16),
        Wl2=np.asarray(Wl2, np.float32).astype(bf16),
        Wr2=np.asarray(Wr2, np.float32).astype(bf16),
        att1m=att1m.astype(bf16), att2m=att2m.astype(bf16),
        bias1r=np.tile(np.asarray(b1, np.float32), (128, 1)),
        bias2r=np.tile(np.asarray(b2, np.float32), (128, 1)),
        id128=np.eye(128, dtype=np.float32),
        iorowb=np.tile(iota, (128, 1)).astype(bf16),
        iocol=iota.reshape(128, 1).copy(),
        iocolb=iota.reshape(128, 1).astype(bf16),
        padmat=padmat,
        e01=np.concatenate([np.ones(H1, np.float32),
                            np.zeros(F, np.float32)]).reshape(1, H1 + F),
        e02=np.concatenate([np.ones(1, np.float32),
                            np.zeros(F, np.float32)]).reshape(1, 1 + F),
    )
    in_maps = []
    for c in range(NC):
        m = dict(shared)
        m["xTloc"] = np.ascontiguousarray(
            xTfull[:, c * NLOCP:(c + 1) * NLOCP]).astype(bf16)
        m.update(core_arrays[c])
        in_maps.append(m)
    meta = dict(pcnt=pcnt, tiles_b=tiles_b, t0_b=t0_b, t1_b=t1_b,
                NT=NT, H1=H1, pcmax=PCMAX)
    return in_maps, meta


# --------------------------------------------------------------------------
# Device program
# --------------------------------------------------------------------------

def build_nc(cfg, meta, profile_nocc=False):
    import concourse.bacc as bacc
    import concourse.tile as tile
    from concourse import mybir

    f32 = mybir.dt.float32
    bf16 = mybir.dt.bfloat16
    i16 = mybir.dt.int16
    AF = mybir.ActivationFunctionType
    OP = mybir.AluOpType

    NC, F = cfg.ncores, cfg.F
    NBLK, NLOCP, NPAD, HALF = cfg.nblk, cfg.nlocp, cfg.npad, cfg.half
    NT, H1 = meta["NT"], meta["H1"]
    tiles_b, t0_b, t1_b = meta["tiles_b"], meta["t0_b"], meta["t1_b"]
    PCMAX = meta["pcmax"]
    TBMAX = PCMAX // 128

    nc = bacc.Bacc("TRN2", target_bir_lowering=False, num_swdge_queues=4)

    din = {}
    def ein(name, shape, dt=f32):
        din[name] = nc.dram_tensor(name, shape, dt, kind="ExternalInput")
        return din[name]

    d_xTfull = ein("xTfull", [128, NPAD], bf16)
    d_xTloc = ein("xTloc", [128, NLOCP], bf16)
    d_Wl1, d_Wr1 = ein("Wl1", [128, 128], bf16), ein("Wr1", [128, 128], bf16)
    d_Wl2, d_Wr2 = ein("Wl2", [128, 128], bf16), ein("Wr2", [128, 128], bf16)
    d_att1 = ein("att1m", [128, H1], bf16)
    d_att2 = ein("att2m", [128, 1], bf16)
    d_b1r, d_b2r = ein("bias1r", [128, F]), ein("bias2r", [128, F])
    d_id = ein("id128", [128, 128])
    d_iorowb = ein("iorowb", [128, 128], bf16)
    d_iocol = ein("iocol", [128, 1])
    d_iocolb = ein("iocolb", [128, 1], bf16)
    d_padm = ein("padmat", [1, NLOCP])
    d_e01, d_e02 = ein("e01", [1, H1 + F]), ein("e02", [1, 1 + F])
    d_idx16 = ein("idx16", [128, NT * 8], i16)
    d_dstcol = ein("dstcol", [128, NT], bf16)
    d_dstrow = ein("dstrow", [NBLK, PCMAX], bf16)

    d_out = nc.dram_tensor("outloc", [NLOCP, F], f32, kind="ExternalOutput")

    d_xl1lo = nc.dram_tensor("xl1lo", [HALF, F], f32)
    d_xl1hi = nc.dram_tensor("xl1hi", [HALF, F], f32)
    d_xl2sh = nc.dram_tensor("xl2sh", [NPAD, F], f32, addr_space="Shared")
    d_xl2loc = nc.dram_tensor("xl2loc", [NLOCP, F], f32)
    if not GATHER_FROM_SHARED:
        d_xl2lo = nc.dram_tensor("xl2lo", [HALF, F], f32)
        d_xl2hi = nc.dram_tensor("xl2hi", [HALF, F], f32)

    with tile.TileContext(nc) as tc:
        with tc.tile_pool(name="const", bufs=1) as cp:
            Wl1_sb = cp.tile_from(d_Wl1[:, :])
            Wr1_sb = cp.tile_from(d_Wr1[:, :])
            Wl2_sb = cp.tile_from(d_Wl2[:, :])
            Wr2_sb = cp.tile_from(d_Wr2[:, :])
            att1_sb = cp.tile_from(d_att1[:, :])
            att2_sb = cp.tile_from(d_att2[:, :])
            b1_sb = cp.tile_from(d_b1r[:, :])
            b2_sb = cp.tile_from(d_b2r[:, :])
            id_sb = cp.tile_from(d_id[:, :])
            iorowb_sb = cp.tile_from(d_iorowb[:, :])
            iocol_sb = cp.tile_from(d_iocol[:, :])
            iocolb_sb = cp.tile_from(d_iocolb[:, :])
            padm_sb = cp.tile_from(d_padm[:, :])
            e01_sb = cp.tile_from(d_e01[:, :])
            e02_sb = cp.tile_from(d_e02[:, :])
            idx_sb = cp.tile_from(d_idx16[:, :])
            dstc_sb = cp.tile_from(d_dstcol[:, :])
            xr1h_sb = cp.tile([128, NLOCP], bf16)
            xr2h_sb = cp.tile([128, NLOCP], bf16)

            # ---------------- phase A: layer-1 tables ----------------
            with (
                tc.tile_pool(name="tabs", bufs=4) as tp,
                tc.tile_pool(name="tabp", bufs=2, space="PSUM") as tpp,
            ):
                for t4 in range(-(-NBLK // 4)):  # local-shard xr1
                    q = min(4, NBLK - 4 * t4)
                    xt = tp.tile([128, 512], bf16, tag="xt")
                    nc.scalar.dma_start(
                        out=xt[:, 0:q * 128],
                        in_=d_xTloc[:, t4 * 512:t4 * 512 + q * 128])
                    for j in range(q):
                        t = 4 * t4 + j
                        ps = tpp.tile([128, 128], f32, tag="psx")
                        nc.tensor.matmul(out=ps[:],
                                         lhsT=xt[:, j * 128:(j + 1) * 128],
                                         rhs=Wr1_sb[:], start=True, stop=True)
                        nc.scalar.copy(xr1h_sb[:, t * 128:(t + 1) * 128], ps[:])
                for t4 in range(NPAD // 512):  # full xl1 table locally
                    ps4 = tpp.tile([128, 512], f32, tag="ps4")
                    xt = tp.tile([128, 512], bf16, tag="xt")
                    nc.scalar.dma_start(out=xt[:],
                                        in_=d_xTfull[:, t4 * 512:(t4 + 1) * 512])
                    for j in range(4):
                        nc.tensor.matmul(out=ps4[:, j * 128:(j + 1) * 128],
                                         lhsT=xt[:, j * 128:(j + 1) * 128],
                                         rhs=Wl1_sb[:],
                                         start=(j == 0), stop=(j == 3),
                                         skip_group_check=True)
                    stg = tp.tile([128, 512], f32, tag="stg")
                    nc.scalar.copy(stg[:], ps4[:])
                    half_t = NPAD // 1024  # 512-row chunks per half table
                    d_tab = d_xl1lo if t4 < half_t else d_xl1hi
                    r0 = (t4 if t4 < half_t else t4 - half_t) * 512
                    nc.sync.dma_start(
                        out=d_tab[r0:r0 + 512, :]
                            .rearrange("(t p) f -> p t f", p=128),
                        in_=stg[:].rearrange("p (t f) -> p t f", t=4),
                    )

            # ---------------- phase B: edge layers ----------------
            def edge_layer(H, d_lo, d_hi, xr_hi, att_sb, e0_sb,
                           bias_sb, epilogue):
                with (
                    tc.tile_pool(name="xp", bufs=3) as xp,
                    tc.tile_pool(name="bp", bufs=3) as bp,
                    tc.tile_pool(name="gp", bufs=4) as gp,
                    tc.tile_pool(name="op", bufs=3) as op,
                    tc.tile_pool(name="epp", bufs=2, space="PSUM") as pp,
                    tc.tile_pool(name="epp1", bufs=1, space="PSUM") as pp1,
                ):
                    ct = 0
                    qrr = [0]
                    for b in range(NBLK):
                        T0, T1 = int(t0_b[b]), int(t1_b[b])
                        TB = T0 + T1
                        EB = TB * 128
                        b_acc = pp.tile([128, H + F], f32, tag="b_acc")
                        nc.tensor.matmul(out=b_acc[:],
                                         lhsT=padm_sb[0:1, b * 128:(b + 1) * 128],
                                         rhs=e0_sb[0:1, 0:H + F],
                                         start=True, stop=False,
                                         skip_group_check=True)
                        drst = bp.tile([1, PCMAX], bf16, tag="drst")
                        nc.scalar.dma_start(out=drst[0:1, 0:EB],
                                            in_=d_dstrow[b:b + 1, 0:EB])
                        # bulk gather of xl[src] for the whole block (one
                        # dma_gather per table half)
                        xl_g = xp.tile([128, TBMAX * F], f32, tag="xl_g")
                        # dma_gather breaks on HW above 1024 indices; use two
                        # balanced chunks per half, spread across the 4 SWDGE
                        # queues so transfers overlap
                        for tab, ta, tb in ((d_lo, 0, T0), (d_hi, T0, TB)):
                            nt = tb - ta
                            for c0, tn in ((ta, (nt + 1) // 2),
                                           (ta + (nt + 1) // 2, nt // 2)):
                                if tn == 0:
                                    continue
                                assert tn <= 8
                                nc.gpsimd.dma_gather(
                                    out_ap=xl_g[:, c0 * F:(c0 + tn) * F]
                                        .rearrange("p (t f) -> p t f", t=tn),
                                    in_ap=tab,
                                    idxs_ap=idx_sb[:, 8 * (ct + c0):
                                                   8 * (ct + c0 + tn)],
                                    num_idxs=tn * 128,
                                    num_idxs_reg=tn * 128,
                                    elem_size=F,
                                    queue_num=qrr[0] % 4,
                                )
                                qrr[0] += 1
                                # queue_num is rewritten post-scheduling to
                                # match the DMASW lane tile actually assigns
                        dstrep = bp.tile([128, PCMAX], bf16, tag="dstrep")
                        if BCAST_VIA_DMA:
                            nc.sync.dma_start(
                                out=dstrep[:, 0:EB],
                                in_=d_dstrow[b:b + 1, 0:EB]
                                    .to_broadcast([128, EB]))
                        else:
                            nc.gpsimd.partition_broadcast(
                                dstrep[:, 0:EB], drst[0:1, 0:EB])
                        m2 = bp.tile([128, PCMAX], bf16, tag="m2")
                        nc.vector.tensor_tensor(
                            out=m2[:, 0:EB], in0=dstrep[:, 0:EB],
                            in1=iocolb_sb[:].to_broadcast([128, EB]),
                            op=OP.is_equal)
                        m = bp.tile([128, PCMAX], bf16, tag="m")
                        nc.vector.tensor_tensor(
                            out=m[:, 0:EB].rearrange("p (t n) -> p t n", t=TB),
                            in0=dstc_sb[:, ct:ct + TB]
                                .unsqueeze(2).to_broadcast([128, TB, 128]),
                            in1=iorowb_sb[:].unsqueeze(1)
                                .to_broadcast([128, TB, 128]),
                            op=OP.is_equal)
                        for g0 in range(0, TB, 4):
                            sz = min(4, TB - g0)
                            E1 = sz * 128
                            o1 = g0 * 128
                            b_et = pp.tile([128, 512], f32, tag="b_et")
                            for j in range(sz):
                                nc.tensor.matmul(
                                    out=b_et[:, j * 128:(j + 1) * 128],
                                    lhsT=xl_g[:, (g0 + j) * F:(g0 + j + 1) * F],
                                    rhs=id_sb[:], is_transpose=True,
                                    start=(j == 0), stop=False,
                                    skip_group_check=True)
                            nc.tensor.matmul(
                                out=b_et[:, 0:E1],
                                lhsT=xr_hi[:, b * 128:(b + 1) * 128],
                                rhs=m2[:, o1:o1 + E1],
                                start=False, stop=True, skip_group_check=True)
                            lrel = gp.tile([128, 512], bf16, tag="lrel")
                            if USE_PRELU:
                                nc.scalar.activation(
                                    out=lrel[:, 0:E1], in_=b_et[:, 0:E1],
                                    func=AF.Prelu, alpha=NEG)
                            else:
                                relu = gp.tile([128, 512], f32, tag="relu")
                                nc.scalar.activation(
                                    out=relu[:, 0:E1], in_=b_et[:, 0:E1],
                                    func=AF.Relu, scale=1.0 - NEG)
                                nc.vector.scalar_tensor_tensor(
                                    out=lrel[:, 0:E1], in0=b_et[:, 0:E1],
                                    scalar=NEG, in1=relu[:, 0:E1],
                                    op0=OP.mult, op1=OP.add)
                            b_s = pp.tile([128, 4 * H], f32, tag="b_s")
                            for j in range(sz):
                                nc.tensor.matmul(
                                    out=b_s[:, j * H:(j + 1) * H],
                                    lhsT=lrel[:, j * 128:(j + 1) * 128],
                                    rhs=att_sb[:, 0:H],
                                    start=(j == 0), stop=(j == sz - 1),
                                    skip_group_check=True)
                            w_sb = gp.tile([128, 4 * (H + F)], bf16, tag="w_sb")
                            nc.scalar.activation(
                                out=w_sb[:].rearrange("p (t x) -> p t x",
                                                      t=4)[:, 0:sz, 0:H],
                                in_=b_s[:, 0:sz * H]
                                    .rearrange("p (t h) -> p t h", t=sz),
                                func=AF.Exp)
                            nc.vector.tensor_tensor(
                                out=w_sb[:].rearrange("p (t x) -> p t x",
                                                      t=4)[:, 0:sz, H:H + F]
                                    .rearrange("p t (h c) -> p t h c", h=H),
                                in0=xl_g[:, g0 * F:(g0 + sz) * F]
                                    .rearrange("p (t h c) -> p t h c",
                                               t=sz, h=H),
                                in1=w_sb[:].rearrange("p (t x) -> p t x",
                                                      t=4)[:, 0:sz, 0:H]
                                    .unsqueeze(3)
                                    .to_broadcast([128, sz, H, F // H]),
                                op=OP.mult)
                            for j in range(sz):
                                nc.tensor.matmul(
                                    out=b_acc[:],
                                    lhsT=m[:, (g0 + j) * 128:(g0 + j + 1) * 128],
                                    rhs=w_sb[:, j * (H + F):(j + 1) * (H + F)],
                                    start=False,
                                    stop=(g0 + sz >= TB and j == sz - 1),
                                    skip_group_check=True)
                        ct += TB
                        # epilogue: divide by denominators, add bias
                        recip = op.tile([128, H], f32, tag="recip")
                        nc.vector.reciprocal(recip[:], b_acc[:, 0:H])
                        outb = op.tile([128, F], f32, tag="outb")
                        C = F // H
                        for h in range(H):
                            nc.vector.tensor_scalar_mul(
                                outb[:, h * C:(h + 1) * C],
                                b_acc[:, H + h * C:H + (h + 1) * C],
                                recip[:, h:h + 1])
                        nc.vector.tensor_tensor(out=outb[:], in0=outb[:],
                                                in1=bias_sb[:], op=OP.add)
                        epilogue(b, outb, op, pp1)

            def epi1(b, outb, wp, pp1):
                ps_h = pp1.tile([128, 128], f32, tag="ps_h")
                nc.tensor.matmul(out=ps_h[:], lhsT=outb[:], rhs=id_sb[:],
                                 is_transpose=True, start=True, stop=True)
                hT = wp.tile([128, 128], bf16, tag="hT")
                nc.scalar.copy(hT[:], ps_h[:])
                ps_x = pp1.tile([128, 128], f32, tag="ps_x2")
                nc.tensor.matmul(out=ps_x[:], lhsT=hT[:], rhs=Wr2_sb[:],
                                 start=True, stop=True)
                nc.scalar.copy(xr2h_sb[:, b * 128:(b + 1) * 128], ps_x[:])
                ps_l = pp1.tile([128, 128], f32, tag="ps_x2")
                nc.tensor.matmul(out=ps_l[:], lhsT=hT[:], rhs=Wl2_sb[:],
                                 start=True, stop=True)
                l2s = wp.tile([128, 128], f32, tag="l2s")
                nc.scalar.copy(l2s[:], ps_l[:])
                nc.sync.dma_start(out=d_xl2loc[b * 128:(b + 1) * 128, :],
                                  in_=l2s[:])

            edge_layer(H1, d_xl1lo[:, :], d_xl1hi[:, :], xr1h_sb,
                       att1_sb, e01_sb, b1_sb, epi1)

            # ------------- phase C: AllGather layer-2 table -------------
            if profile_nocc:
                for s in range(NC):
                    nc.sync.dma_start(
                        out=d_xl2sh[s * NLOCP:(s + 1) * NLOCP, :],
                        in_=d_xl2loc[:, :])
            else:
                nc.gpsimd.collective_compute(
                    "AllGather", mybir.AluOpType.bypass,
                    replica_groups=[list(range(NC))],
                    ins=[d_xl2loc[:, :]], outs=[d_xl2sh[:, :]],
                )
            if GATHER_FROM_SHARED:
                d_l2lo, d_l2hi = d_xl2sh[0:HALF, :], d_xl2sh[HALF:NPAD, :]
            else:
                nc.sync.dma_start(out=d_xl2lo[:, :], in_=d_xl2sh[0:HALF, :])
                nc.scalar.dma_start(out=d_xl2hi[:, :],
                                    in_=d_xl2sh[HALF:NPAD, :])
                d_l2lo, d_l2hi = d_xl2lo[:, :], d_xl2hi[:, :]

            # ---------------- phase E: layer-2 edges ----------------
            def epi2(b, outb, wp, pp1):
                nc.sync.dma_start(out=d_out[b * 128:(b + 1) * 128, :],
                                  in_=outb[:])

            edge_layer(1, d_l2lo, d_l2hi, xr2h_sb, att2_sb,
                       e02_sb, b2_sb, epi2)

    # The SWDGE ucode locks each semaphore to one queue, but tile's DMASW
    # lane rotation follows scheduled order (which can differ from emission
    # order). Re-derive queue_num from the lane tile actually assigned so
    # lane <-> queue is a pure function (lane % 4).
    for f in nc.m.functions:
        for blk in f.blocks:
            for inst in blk.instructions:
                if type(inst).__name__ == "InstDMAGatherAnt":
                    si = inst.sync_info
                    for u in (si.on_update or []) if si else []:
                        nmu = str(getattr(u, "ant_name", "") or "")
                        if nmu.startswith("DMASW"):
                            inst.queue_num = int(nmu.split("_")[0][5:]) % 4

    nc.compile()
    return nc


# --------------------------------------------------------------------------
# Entry point
# --------------------------------------------------------------------------

_NC_CACHE = {}


def kernel(x, edge_index, edge_attr, Wl1, Wr1, att1, b1, Wl2, Wr2, att2, b2,
           cfg=None, _want_results=False):
    from concourse.bass_utils import run_bass_kernel_spmd

    cfg = cfg or CFG
    in_maps, meta = host_prep(x, edge_index, Wl1, Wr1, att1, b1,
                              Wl2, Wr2, att2, b2, cfg)
    key = (cfg.N, cfg.E, tuple(meta["pcnt"].tolist()))
    nc = _NC_CACHE.get(key)
    if nc is None:
        nc = build_nc(cfg, meta)
        _NC_CACHE[key] = nc
    res = run_bass_kernel_spmd(nc, in_maps, core_ids=list(range(cfg.ncores)))
    out = np.empty((cfg.N, cfg.F), dtype=np.float32)
    for c in range(cfg.ncores):
        out[c * cfg.nloc:(c + 1) * cfg.nloc] = \
            res.results[c]["outloc"][:cfg.nloc]
    if _want_results:
        return out, res
    return out


# revision 44
# speedup vs baseline: 2.4257x; 1.0323x over previous
"""GATv2 (2-layer, N=50000, E=800000) on 8 Trainium2 NeuronCores.

Strategy (self-contained; shapes hardcoded for nn_GATUnit_34067680592302):
  - Nodes partitioned across 8 cores (6250 each, padded to 6272 = 49 blocks
    of 128). Edges (incl. self-loops) assigned by destination node and sorted
    by destination, so scatter-softmax / segment-sum stay core-local.
  - Per layer, every core holds the full "source transform" table
    xl = x @ Wl in its DRAM (layer 1: computed locally from replicated x;
    layer 2: local h shard transformed then AllGather'ed), and bulk-gathers
    xl[src] rows per destination block with ONE dma_gather per block-half
    (int16 indices limit a gather table to 32K rows, so the table is split
    in two 25088-row halves and each block's edge list is stored as
    [half0-edges | pad | half1-edges | pad], each padded to a 128 multiple
    with index-0 rows that are masked out via dst=-1).
  - Per 128-node block, attention + weighted aggregation accumulate in PSUM
    via selection-matrix matmuls; a final reciprocal-scale epilogue divides
    by the softmax denominators (max-subtraction is skipped: |s| <= ~10 so
    exp() is safe in fp32).
"""
import sys
sys.path.insert(0, "/opt/trn_rl_repo")

import numpy as np

NEG = 0.2
USE_PRELU = True  # HW Prelu == leaky_relu(x, alpha); CoreSim lacks it
GATHER_FROM_SHARED = True  # layer-2 gathers read the AllGather buf directly
BCAST_VIA_DMA = True  # dstrep via sync-DMA broadcast (else gpsimd ucode)


class Cfg:
    def __init__(self, N=50000, E=800000, ncores=8, nloc=6250, F=128):
        assert N == ncores * nloc
        self.N, self.E, self.ncores, self.nloc, self.F = N, E, ncores, nloc, F
        self.nblk = -(-nloc // 128)          # blocks of 128 nodes per core
        self.nlocp = self.nblk * 128         # padded local nodes
        self.npad = ncores * self.nlocp      # padded global nodes
        self.half = self.npad // 2           # gather-table half size (<32K)
        assert self.npad % 1024 == 0 and self.half < 32768


CFG = Cfg()


# --------------------------------------------------------------------------
# Host-side preprocessing
# --------------------------------------------------------------------------

def host_prep(x, edge_index, Wl1, Wr1, att1, b1, Wl2, Wr2, att2, b2, cfg):
    N, E, NC, NLOC = cfg.N, cfg.E, cfg.ncores, cfg.nloc
    NBLK, NLOCP, NPAD, F = cfg.nblk, cfg.nlocp, cfg.npad, cfg.F
    HALF = cfg.half

    src0 = np.asarray(edge_index[0]).astype(np.int64)
    dst0 = np.asarray(edge_index[1]).astype(np.int64)
    loops = np.arange(N, dtype=np.int64)
    SRC = np.concatenate([src0, loops])
    DST = np.concatenate([dst0, loops])
    shard = DST // NLOC
    src_g = ((SRC // NLOC) * NLOCP + (SRC % NLOC)).astype(np.int32)
    dst_loc = (DST - shard * NLOC).astype(np.int64)

    per_core = []
    n_lo = np.zeros((NC, NBLK), dtype=np.int64)
    n_hi = np.zeros((NC, NBLK), dtype=np.int64)
    for c in range(NC):
        sel = shard == c
        sg, dl = src_g[sel], dst_loc[sel]
        order = np.argsort(dl, kind="stable")
        sg, dl = sg[order], dl[order]
        blk = dl // 128
        lo = sg < HALF
        per_core.append((sg, dl, blk, lo))
        n_lo[c] = np.bincount(blk[lo], minlength=NBLK)
        n_hi[c] = np.bincount(blk[~lo], minlength=NBLK)

    # +2 slots per half: a dummy valid index at the head of each gather
    # chunk (ensures every chunk has >=1 valid index; the rest of the pad
    # is idx=-1, which dma_gather skips entirely given a runtime count)
    t0_b = (-(-(n_lo.max(axis=0) + 2) // 128)).astype(np.int64)
    t1_b = (-(-(n_hi.max(axis=0) + 2) // 128)).astype(np.int64)
    tiles_b = t0_b + t1_b
    pcnt = tiles_b * 128
    offs = np.concatenate([[0], np.cumsum(pcnt)])
    NTOT = int(offs[-1])
    NT = NTOT // 128
    PCMAX = int(pcnt.max())

    # chunk table (shared by both layers): per block-half, two gather
    # chunks [A | B] each <= 8 tiles, dummy at each chunk head
    chunks = []  # (tile_offset_global, tn)
    chunk_of = []  # per block: list of (slot_lo, slot_hi) chunk index range
    for b in range(NBLK):
        ct = int(offs[b]) // 128
        lst = []
        for ta, t in ((ct, int(t0_b[b])), (ct + int(t0_b[b]), int(t1_b[b]))):
            tA = (t + 1) // 2
            tB = t - tA
            segs = [(ta, tA)] + ([(ta + tA, tB)] if tB else [])
            lst.append([len(chunks) + i for i in range(len(segs))])
            chunks.extend(segs)
        chunk_of.append(lst)
    NCH = len(chunks)
    assert all(tn <= 8 for _, tn in chunks)

    import ml_dtypes
    bf16 = ml_dtypes.bfloat16

    core_arrays = []
    for c in range(NC):
        sg, dl, blk, lo = per_core[c]
        idx_arr = np.full(NTOT, -1, dtype=np.int16)
        dst_arr = np.full(NTOT, -1.0, dtype=np.float32)
        cnts = np.zeros(NCH, dtype=np.int32)
        for b in range(NBLK):
            selb = blk == b
            sgb, dlb, lob = sg[selb], dl[selb] - 128 * b, lo[selb]
            for half, (sgh, dlh) in enumerate(
                    ((sgb[lob], dlb[lob]),
                     (sgb[~lob] - HALF, dlb[~lob]))):
                n = len(sgh)
                p = 0  # consumed edges
                for k in chunk_of[b][half]:
                    t0c, tn = chunks[k]
                    o = t0c * 128
                    idx_arr[o] = 0          # dummy valid index
                    dst_arr[o] = -1.0
                    take = min(n - p, tn * 128 - 1)
                    idx_arr[o + 1:o + 1 + take] = sgh[p:p + take]
                    dst_arr[o + 1:o + 1 + take] = dlh[p:p + take]
                    cnts[k] = take + 1
                    p += take
                assert p == n
        # runtime-count gathers (idx=-1 suffix skipping) crash the HW
        # ucode; gather every slot (pad slots fetch row 0, masked by dst=-1)
        idx_arr[idx_arr < 0] = 0
        # dma_gather wraps index j to [j % 16, j // 16]; replicate x8 rows
        idx16 = np.tile(
            np.ascontiguousarray(idx_arr.reshape(-1, 16).T), (8, 1))
        dstcol = np.ascontiguousarray(
            dst_arr.reshape(NT, 128).T).astype(bf16)
        dstrow = np.full((NBLK, PCMAX), -1.0, dtype=np.float32)
        for b in range(NBLK):
            o = int(offs[b])
            dstrow[b, :int(pcnt[b])] = dst_arr[o:o + int(pcnt[b])]
        core_arrays.append(dict(idx16=idx16, dstcol=dstcol,
                                dstrow=dstrow.astype(bf16),
                                gcnt=cnts.reshape(1, NCH)))

    # node features, transposed + padded: xT[f, g] with g = s*NLOCP + j
    x = np.asarray(x, dtype=np.float32)
    xpad = np.zeros((NPAD, F), dtype=np.float32)
    for s in range(NC):
        xpad[s * NLOCP:s * NLOCP + NLOC] = x[s * NLOC:(s + 1) * NLOC]
    xTfull = np.ascontiguousarray(xpad.T)

    H1 = att1.shape[0]
    C1 = att1.shape[1]
    att1m = np.zeros((128, H1), dtype=np.float32)
    for h in range(H1):
        att1m[h * C1:(h + 1) * C1, h] = att1[h]
    att2m = np.zeros((128, 1), dtype=np.float32)
    att2m[:att2.shape[1], 0] = att2[0]

    iota = np.arange(128, dtype=np.float32)
    padmat = (np.arange(NLOCP) >= NLOC).astype(np.float32).reshape(1, NLOCP)

    shared = dict(
        xTfull=xTfull.astype(bf16),
        Wl1=np.asarray(Wl1, np.float32).astype(bf16),
        Wr1=np.asarray(Wr1, np.float32).astype(bf16),
        Wl2=np.asarray(Wl2, np.float32).astype(bf16),
        Wr2=np.asarray(Wr2, np.float32).astype(bf16),
        att1m=att1m.astype(bf16), att2m=att2m.astype(bf16),
        bias1r=np.tile(np.asarray(b1, np.float32), (128, 1)),
        bias2r=np.tile(np.asarray(b2, np.float32), (128, 1)),
        id128=np.eye(128, dtype=np.float32),
        iorowb=np.tile(iota, (128, 1)).astype(bf16),
        iocol=iota.reshape(128, 1).copy(),
        iocolb=iota.reshape(128, 1).astype(bf16),
        padmat=padmat,
        e01=np.concatenate([np.ones(H1, np.float32),
                            np.zeros(F, np.float32)]).reshape(1, H1 + F),
        e02=np.concatenate([np.ones(1, np.float32),
                            np.zeros(F, np.float32)]).reshape(1, 1 + F),
    )
    in_maps = []
    for c in range(NC):
        m = dict(shared)
        m["xTloc"] = np.ascontiguousarray(
            xTfull[:, c * NLOCP:(c + 1) * NLOCP]).astype(bf16)
        m.update(core_arrays[c])
        in_maps.append(m)
    meta = dict(pcnt=pcnt, tiles_b=tiles_b, t0_b=t0_b, t1_b=t1_b,
                NT=NT, H1=H1, pcmax=PCMAX, chunks=chunks,
                chunk_of=chunk_of, nch=NCH)
    return in_maps, meta


# --------------------------------------------------------------------------
# Device program
# --------------------------------------------------------------------------

def build_nc(cfg, meta, profile_nocc=False, zero_xlg=False):
    # zero_xlg: CoreSim models every tile allocation as fresh NaN memory, so
    # for sim validation every block's xl_g is zeroed. On HW the ring reuses
    # the same finite-valued SBUF, so zeroing the first ring pass suffices
    # (pad slots are masked out via dst=-1 regardless of value).
    import concourse.bacc as bacc
    import concourse.tile as tile
    from concourse import mybir

    f32 = mybir.dt.float32
    bf16 = mybir.dt.bfloat16
    i16 = mybir.dt.int16
    AF = mybir.ActivationFunctionType
    OP = mybir.AluOpType

    NC, F = cfg.ncores, cfg.F
    NBLK, NLOCP, NPAD, HALF = cfg.nblk, cfg.nlocp, cfg.npad, cfg.half
    NT, H1 = meta["NT"], meta["H1"]
    tiles_b, t0_b, t1_b = meta["tiles_b"], meta["t0_b"], meta["t1_b"]
    PCMAX = meta["pcmax"]
    TBMAX = PCMAX // 128
    chunks, chunk_of, NCH = meta["chunks"], meta["chunk_of"], meta["nch"]

    nc = bacc.Bacc("TRN2", target_bir_lowering=False, num_swdge_queues=4)

    din = {}
    def ein(name, shape, dt=f32):
        din[name] = nc.dram_tensor(name, shape, dt, kind="ExternalInput")
        return din[name]

    d_xTfull = ein("xTfull", [128, NPAD], bf16)
    d_xTloc = ein("xTloc", [128, NLOCP], bf16)
    d_Wl1, d_Wr1 = ein("Wl1", [128, 128], bf16), ein("Wr1", [128, 128], bf16)
    d_Wl2, d_Wr2 = ein("Wl2", [128, 128], bf16), ein("Wr2", [128, 128], bf16)
    d_att1 = ein("att1m", [128, H1], bf16)
    d_att2 = ein("att2m", [128, 1], bf16)
    d_b1r, d_b2r = ein("bias1r", [128, F]), ein("bias2r", [128, F])
    d_id = ein("id128", [128, 128])
    d_iorowb = ein("iorowb", [128, 128], bf16)
    d_iocol = ein("iocol", [128, 1])
    d_iocolb = ein("iocolb", [128, 1], bf16)
    d_padm = ein("padmat", [1, NLOCP])
    d_e01, d_e02 = ein("e01", [1, H1 + F]), ein("e02", [1, 1 + F])
    d_idx16 = ein("idx16", [128, NT * 8], i16)
    d_dstcol = ein("dstcol", [128, NT], bf16)
    d_dstrow = ein("dstrow", [NBLK, PCMAX], bf16)
    d_gcnt = ein("gcnt", [1, NCH], mybir.dt.int32)

    d_out = nc.dram_tensor("outloc", [NLOCP, F], f32, kind="ExternalOutput")

    d_xl1lo = nc.dram_tensor("xl1lo", [HALF, F], f32)
    d_xl1hi = nc.dram_tensor("xl1hi", [HALF, F], f32)
    d_xl2sh = nc.dram_tensor("xl2sh", [NPAD, F], f32, addr_space="Shared")
    d_xl2loc = nc.dram_tensor("xl2loc", [NLOCP, F], f32)
    if not GATHER_FROM_SHARED:
        d_xl2lo = nc.dram_tensor("xl2lo", [HALF, F], f32)
        d_xl2hi = nc.dram_tensor("xl2hi", [HALF, F], f32)

    with tile.TileContext(nc) as tc:
        with tc.tile_pool(name="const", bufs=1) as cp:
            Wl1_sb = cp.tile_from(d_Wl1[:, :])
            Wr1_sb = cp.tile_from(d_Wr1[:, :])
            Wl2_sb = cp.tile_from(d_Wl2[:, :])
            Wr2_sb = cp.tile_from(d_Wr2[:, :])
            att1_sb = cp.tile_from(d_att1[:, :])
            att2_sb = cp.tile_from(d_att2[:, :])
            b1_sb = cp.tile_from(d_b1r[:, :])
            b2_sb = cp.tile_from(d_b2r[:, :])
            id_sb = cp.tile_from(d_id[:, :])
            iorowb_sb = cp.tile_from(d_iorowb[:, :])
            iocol_sb = cp.tile_from(d_iocol[:, :])
            iocolb_sb = cp.tile_from(d_iocolb[:, :])
            padm_sb = cp.tile_from(d_padm[:, :])
            e01_sb = cp.tile_from(d_e01[:, :])
            e02_sb = cp.tile_from(d_e02[:, :])
            idx_sb = cp.tile_from(d_idx16[:, :])
            dstc_sb = cp.tile_from(d_dstcol[:, :])
            gcnt_sb = cp.tile_from(d_gcnt[:, :])
            xr1h_sb = cp.tile([128, NLOCP], bf16)
            xr2h_sb = cp.tile([128, NLOCP], bf16)

            # ---------------- phase A: layer-1 tables ----------------
            with (
                tc.tile_pool(name="tabs", bufs=4) as tp,
                tc.tile_pool(name="tabp", bufs=2, space="PSUM") as tpp,
            ):
                for t4 in range(-(-NBLK // 4)):  # local-shard xr1
                    q = min(4, NBLK - 4 * t4)
                    xt = tp.tile([128, 512], bf16, tag="xt")
                    nc.sync.dma_start(
                        out=xt[:, 0:q * 128],
                        in_=d_xTloc[:, t4 * 512:t4 * 512 + q * 128])
                    for j in range(q):
                        t = 4 * t4 + j
                        ps = tpp.tile([128, 128], f32, tag="psx")
                        nc.tensor.matmul(out=ps[:],
                                         lhsT=xt[:, j * 128:(j + 1) * 128],
                                         rhs=Wr1_sb[:], start=True, stop=True)
                        nc.scalar.copy(xr1h_sb[:, t * 128:(t + 1) * 128], ps[:])
                for t4 in range(NPAD // 512):  # full xl1 table locally
                    ps4 = tpp.tile([128, 512], f32, tag="ps4")
                    xt = tp.tile([128, 512], bf16, tag="xt")
                    nc.sync.dma_start(out=xt[:],
                                      in_=d_xTfull[:, t4 * 512:(t4 + 1) * 512])
                    for j in range(4):
                        nc.tensor.matmul(out=ps4[:, j * 128:(j + 1) * 128],
                                         lhsT=xt[:, j * 128:(j + 1) * 128],
                                         rhs=Wl1_sb[:],
                                         start=(j == 0), stop=(j == 3),
                                         skip_group_check=True)
                    stg = tp.tile([128, 512], f32, tag="stg")
                    nc.scalar.copy(stg[:], ps4[:])
                    half_t = NPAD // 1024  # 512-row chunks per half table
                    d_tab = d_xl1lo if t4 < half_t else d_xl1hi
                    r0 = (t4 if t4 < half_t else t4 - half_t) * 512
                    nc.sync.dma_start(
                        out=d_tab[r0:r0 + 512, :]
                            .rearrange("(t p) f -> p t f", p=128),
                        in_=stg[:].rearrange("p (t f) -> p t f", t=4),
                    )

            # ---------------- phase B: edge layers ----------------
            def edge_layer(H, d_lo, d_hi, xr_hi, att_sb, e0_sb,
                           bias_sb, epilogue):
                with (
                    tc.tile_pool(name="xp", bufs=4) as xp,
                    tc.tile_pool(name="bp", bufs=3) as bp,
                    tc.tile_pool(name="gp", bufs=4) as gp,
                    tc.tile_pool(name="op", bufs=3) as op,
                    tc.tile_pool(name="epp", bufs=2, space="PSUM") as pp,
                    tc.tile_pool(name="epp1", bufs=1, space="PSUM") as pp1,
                ):
                    ct = 0
                    qrr = [0]
                    for b in range(NBLK):
                        T0, T1 = int(t0_b[b]), int(t1_b[b])
                        TB = T0 + T1
                        EB = TB * 128
                        b_acc = pp.tile([128, H + F], f32, tag="b_acc")
                        nc.tensor.matmul(out=b_acc[:],
                                         lhsT=padm_sb[0:1, b * 128:(b + 1) * 128],
                                         rhs=e0_sb[0:1, 0:H + F],
                                         start=True, stop=False,
                                         skip_group_check=True)
                        drst = bp.tile([1, PCMAX], bf16, tag="drst")
                        nc.sync.dma_start(out=drst[0:1, 0:EB],
                                          in_=d_dstrow[b:b + 1, 0:EB])
                        # bulk gather of xl[src] for the whole block: one
                        # dma_gather per chunk (<=1024 idxs; HW breaks above),
                        # spread across the 4 SWDGE queues. Pad slots carry
                        # idx=-1 and are skipped via the runtime count; the
                        # stale xl_g contents they leave are finite (buffers
                        # are zeroed on first use) and masked out by dst=-1.
                        xl_g = xp.tile([128, TBMAX * F], f32, tag="xl_g")
                        if zero_xlg or b < 4:
                            nc.vector.memset(xl_g[:], 0.0)
                        for half in (0, 1):
                            tab = d_lo if half == 0 else d_hi
                            for k in chunk_of[b][half]:
                                c0g, tn = chunks[k]
                                nc.gpsimd.dma_gather(
                                    out_ap=xl_g[:, (c0g - ct) * F:
                                                (c0g - ct + tn) * F]
                                        .rearrange("p (t f) -> p t f", t=tn),
                                    in_ap=tab,
                                    idxs_ap=idx_sb[:, 8 * c0g:8 * (c0g + tn)],
                                    num_idxs=tn * 128,
                                    num_idxs_reg=tn * 128,
                                    elem_size=F,
                                    queue_num=qrr[0] % 4,
                                )
                                qrr[0] += 1
                                # queue_num is rewritten post-scheduling to
                                # match the DMASW lane tile actually assigns
                        dstrep = bp.tile([128, PCMAX], bf16, tag="dstrep")
                        if BCAST_VIA_DMA:
                            nc.sync.dma_start(
                                out=dstrep[:, 0:EB],
                                in_=d_dstrow[b:b + 1, 0:EB]
                                    .to_broadcast([128, EB]))
                        else:
                            nc.gpsimd.partition_broadcast(
                                dstrep[:, 0:EB], drst[0:1, 0:EB])
                        m2 = bp.tile([128, PCMAX], bf16, tag="m2")
                        nc.vector.tensor_tensor(
                            out=m2[:, 0:EB], in0=dstrep[:, 0:EB],
                            in1=iocolb_sb[:].to_broadcast([128, EB]),
                            op=OP.is_equal)
                        m = bp.tile([128, PCMAX], bf16, tag="m")
                        nc.vector.tensor_tensor(
                            out=m[:, 0:EB].rearrange("p (t n) -> p t n", t=TB),
                            in0=dstc_sb[:, ct:ct + TB]
                                .unsqueeze(2).to_broadcast([128, TB, 128]),
                            in1=iorowb_sb[:].unsqueeze(1)
                                .to_broadcast([128, TB, 128]),
                            op=OP.is_equal)
                        def stage2(g0, sz, lrel):
                            # score, softmax weight, weighted accumulation for
                            # a group; runs one group behind stage 1 so the
                            # in-order engine queues never stall on results
                            # that were just produced by another engine
                            b_s = pp.tile([128, 4 * H], f32, tag="b_s")
                            for j in range(sz):
                                nc.tensor.matmul(
                                    out=b_s[:, j * H:(j + 1) * H],
                                    lhsT=lrel[:, j * 128:(j + 1) * 128],
                                    rhs=att_sb[:, 0:H],
                                    start=(j == 0), stop=(j == sz - 1),
                                    skip_group_check=True)
                            w_sb = gp.tile([128, 4 * (H + F)], bf16,
                                           tag="w_sb")
                            nc.scalar.activation(
                                out=w_sb[:].rearrange("p (t x) -> p t x",
                                                      t=4)[:, 0:sz, 0:H],
                                in_=b_s[:, 0:sz * H]
                                    .rearrange("p (t h) -> p t h", t=sz),
                                func=AF.Exp)
                            nc.vector.tensor_tensor(
                                out=w_sb[:].rearrange("p (t x) -> p t x",
                                                      t=4)[:, 0:sz, H:H + F]
                                    .rearrange("p t (h c) -> p t h c", h=H),
                                in0=xl_g[:, g0 * F:(g0 + sz) * F]
                                    .rearrange("p (t h c) -> p t h c",
                                               t=sz, h=H),
                                in1=w_sb[:].rearrange("p (t x) -> p t x",
                                                      t=4)[:, 0:sz, 0:H]
                                    .unsqueeze(3)
                                    .to_broadcast([128, sz, H, F // H]),
                                op=OP.mult)
                            for j in range(sz):
                                nc.tensor.matmul(
                                    out=b_acc[:],
                                    lhsT=m[:, (g0 + j) * 128:
                                           (g0 + j + 1) * 128],
                                    rhs=w_sb[:, j * (H + F):(j + 1) * (H + F)],
                                    start=False,
                                    stop=(g0 + sz >= TB and j == sz - 1),
                                    skip_group_check=True)

                        pend = None
                        for g0 in range(0, TB, 4):
                            sz = min(4, TB - g0)
                            E1 = sz * 128
                            o1 = g0 * 128
                            b_et = pp.tile([128, 512], f32, tag="b_et")
                            for j in range(sz):
                                nc.tensor.matmul(
                                    out=b_et[:, j * 128:(j + 1) * 128],
                                    lhsT=xl_g[:, (g0 + j) * F:(g0 + j + 1) * F],
                                    rhs=id_sb[:], is_transpose=True,
                                    start=(j == 0), stop=False,
                                    skip_group_check=True)
                            nc.tensor.matmul(
                                out=b_et[:, 0:E1],
                                lhsT=xr_hi[:, b * 128:(b + 1) * 128],
                                rhs=m2[:, o1:o1 + E1],
                                start=False, stop=True, skip_group_check=True)
                            lrel = gp.tile([128, 512], bf16, tag="lrel")
                            if USE_PRELU:
                                nc.scalar.activation(
                                    out=lrel[:, 0:E1], in_=b_et[:, 0:E1],
                                    func=AF.Prelu, alpha=NEG)
                            else:
                                relu = gp.tile([128, 512], f32, tag="relu")
                                nc.scalar.activation(
                                    out=relu[:, 0:E1], in_=b_et[:, 0:E1],
                                    func=AF.Relu, scale=1.0 - NEG)
                                nc.vector.scalar_tensor_tensor(
                                    out=lrel[:, 0:E1], in0=b_et[:, 0:E1],
                                    scalar=NEG, in1=relu[:, 0:E1],
                                    op0=OP.mult, op1=OP.add)
                            if pend is not None:
                                stage2(*pend)
                            pend = (g0, sz, lrel)
                        stage2(*pend)
                        ct += TB
                        # epilogue: divide by denominators, add bias
                        recip = op.tile([128, H], f32, tag="recip")
                        nc.vector.reciprocal(recip[:], b_acc[:, 0:H])
                        outb = op.tile([128, F], f32, tag="outb")
                        C = F // H
                        for h in range(H):
                            nc.vector.tensor_scalar_mul(
                                outb[:, h * C:(h + 1) * C],
                                b_acc[:, H + h * C:H + (h + 1) * C],
                                recip[:, h:h + 1])
                        nc.vector.tensor_tensor(out=outb[:], in0=outb[:],
                                                in1=bias_sb[:], op=OP.add)
                        epilogue(b, outb, op, pp1)

            def epi1(b, outb, wp, pp1):
                ps_h = pp1.tile([128, 128], f32, tag="ps_h")
                nc.tensor.matmul(out=ps_h[:], lhsT=outb[:], rhs=id_sb[:],
                                 is_transpose=True, start=True, stop=True)
                hT = wp.tile([128, 128], bf16, tag="hT")
                nc.scalar.copy(hT[:], ps_h[:])
                ps_x = pp1.tile([128, 128], f32, tag="ps_x2")
                nc.tensor.matmul(out=ps_x[:], lhsT=hT[:], rhs=Wr2_sb[:],
                                 start=True, stop=True)
                nc.scalar.copy(xr2h_sb[:, b * 128:(b + 1) * 128], ps_x[:])
                ps_l = pp1.tile([128, 128], f32, tag="ps_x2")
                nc.tensor.matmul(out=ps_l[:], lhsT=hT[:], rhs=Wl2_sb[:],
                                 start=True, stop=True)
                l2s = wp.tile([128, 128], f32, tag="l2s")
                nc.scalar.copy(l2s[:], ps_l[:])
                nc.sync.dma_start(out=d_xl2loc[b * 128:(b + 1) * 128, :],
                                  in_=l2s[:])

            edge_layer(H1, d_xl1lo[:, :], d_xl1hi[:, :], xr1h_sb,
                       att1_sb, e01_sb, b1_sb, epi1)

            # ------------- phase C: AllGather layer-2 table -------------
            if profile_nocc:
                for s in range(NC):
                    nc.sync.dma_start(
                        out=d_xl2sh[s * NLOCP:(s + 1) * NLOCP, :],
                        in_=d_xl2loc[:, :])
            else:
                nc.gpsimd.collective_compute(
                    "AllGather", mybir.AluOpType.bypass,
                    replica_groups=[list(range(NC))],
                    ins=[d_xl2loc[:, :]], outs=[d_xl2sh[:, :]],
                )
            if GATHER_FROM_SHARED:
                d_l2lo, d_l2hi = d_xl2sh[0:HALF, :], d_xl2sh[HALF:NPAD, :]
            else:
                nc.sync.dma_start(out=d_xl2lo[:, :], in_=d_xl2sh[0:HALF, :])
                nc.scalar.dma_start(out=d_xl2hi[:, :],
                                    in_=d_xl2sh[HALF:NPAD, :])
                d_l2lo, d_l2hi = d_xl2lo[:, :], d_xl2hi[:, :]

            # ---------------- phase E: layer-2 edges ----------------
            def epi2(b, outb, wp, pp1):
                nc.sync.dma_start(out=d_out[b * 128:(b + 1) * 128, :],
                                  in_=outb[:])

            edge_layer(1, d_l2lo, d_l2hi, xr2h_sb, att2_sb,
                       e02_sb, b2_sb, epi2)

    # The SWDGE ucode locks each semaphore to one queue, but tile's DMASW
    # lane rotation follows scheduled order (which can differ from emission
    # order). Re-derive queue_num from the lane tile actually assigned so
    # lane <-> queue is a pure function (lane % 4).
    for f in nc.m.functions:
        for blk in f.blocks:
            for inst in blk.instructions:
                if type(inst).__name__ == "InstDMAGatherAnt":
                    si = inst.sync_info
                    for u in (si.on_update or []) if si else []:
                        nmu = str(getattr(u, "ant_name", "") or "")
                        if nmu.startswith("DMASW"):
                            inst.queue_num = int(nmu.split("_")[0][5:]) % 4

    nc.compile()
    return nc


# --------------------------------------------------------------------------
# Entry point
# --------------------------------------------------------------------------

_NC_CACHE = {}


def kernel(x, edge_index, edge_attr, Wl1, Wr1, att1, b1, Wl2, Wr2, att2, b2,
           cfg=None, _want_results=False):
    from concourse.bass_utils import run_bass_kernel_spmd

    cfg = cfg or CFG
    in_maps, meta = host_prep(x, edge_index, Wl1, Wr1, att1, b1,
                              Wl2, Wr2, att2, b2, cfg)
    key = (cfg.N, cfg.E, tuple(meta["pcnt"].tolist()))
    nc = _NC_CACHE.get(key)
    if nc is None:
        nc = build_nc(cfg, meta)
        _NC_CACHE[key] = nc
    res = run_bass_kernel_spmd(nc, in_maps, core_ids=list(range(cfg.ncores)))
    out = np.empty((cfg.N, cfg.F), dtype=np.float32)
    for c in range(cfg.ncores):
        out[c * cfg.nloc:(c + 1) * cfg.nloc] = \
            res.results[c]["outloc"][:cfg.nloc]
    if _want_results:
        return out, res
    return out
